# revision 36
# baseline (speedup 1.0000x reference)
"""Trainium2 Bass kernel for nn_MultiHeadAttention_63015760167496.

Computation (see reference): qkv = x @ Wqkv; RoPE on q,k; causal softmax
attention per head; out = einsum('bhts,bshd->bhtd', probs, v);
out.reshape(B,T,C) @ Wout  -- the reshape is a *head-major* flatten of
[B,H,T,D] into [B,T,C], so final-output row r = h*128 + t//16 depends only
on head h.  Sharding: head-parallel over 8 cores (2 heads/core); every core
computes its two heads end-to-end and produces final-output rows
[256*i, 256*i+256).  Host concatenates -- no collectives.

All on-device data is bf16 (PSUM accumulation f32), which halves DMA/SBUF
vs f32r at the same 1 cycle/row PE rate.  Attention runs in S^T layout
([s,t]): softmax denominator via a ones-column matmul (partition reduction
on the PE), normalization broadcast via gpsimd.partition_broadcast and a
flat VE multiply, both issued *deferred* (inside the next t-tile's block
loop) so they never gate the PE.  The attention inner loop is
software-pipelined: score blocks are processed in pairs sharing one
two-bank PSUM tile and a single exp instruction, and the PV/sum matmuls of
the previous two pairs are flushed in same-accumulation-group bursts (a
LDWEIGHTS after an accumulating matmul whose group is suspended stalls
~95ns on hw).  O^T is stored flat [d, t]; the out-projection reads it
through a strided LDWEIGHTS view.  Wout is prefetched into SBUF during
attention b=0 so the out-projection phases are pure PE.
"""

import math
import sys

for _p in ("/opt/trn_rl_repo", "/root/.axon_site/_ro/trn_rl_repo"):
    if _p not in sys.path:
        sys.path.insert(0, _p)

import numpy as np
import ml_dtypes

import concourse.bass as bass
import concourse.mybir as mybir
import concourse.tile as tile
from concourse import bacc
from concourse.bass_utils import run_bass_kernel_spmd

B, T, C = 2, 2048, 2048
H = 16            # heads total
D = C // H        # 128 head dim
HALF = D // 2     # 64
P = 128
KO = C // P       # 16 contraction chunks
NCORES = 8
HPC = H // NCORES  # 2 heads per core
TQ = 512          # t-tile for qkv projection
NT = T // TQ      # 4
TA = 512          # t-tile for attention
NTA = T // TA     # 4
NSC = T // P      # 16 s-chunks
ROPE_BASE = 10000.0
SCALE = 1.0 / math.sqrt(D)
TC_ = 512         # col-tile for out projection
NCP = C // TC_    # 4
LOOKAHEAD = 2     # attention software-pipeline depth

f32 = mybir.dt.float32
bf16 = mybir.dt.bfloat16
EXP = mybir.ActivationFunctionType.Exp


def _build():
    nc = bacc.Bacc("TRN2", target_bir_lowering=False, debug=False,
                   num_devices=NCORES)

    # host-pre-tiled x^T: xTt[b, ti, p, ko, u] = x[b, ti*TQ+u, ko*128+p]
    xTt = nc.dram_tensor("xTt", [B, NT, P, KO, TQ], bf16, kind="ExternalInput")
    # host-pre-chunked weights: w[p, ko, m] = W[ko*128+p, m]
    wq = nc.dram_tensor("wq", [P, KO, HPC * D], bf16, kind="ExternalInput")
    wk = nc.dram_tensor("wk", [P, KO, HPC * D], bf16, kind="ExternalInput")
    wv = nc.dram_tensor("wv", [P, KO, HPC * D], bf16, kind="ExternalInput")
    # woutT[j, p, c] = Wout[j*128+p, c]
    woutT = nc.dram_tensor("woutT", [KO, P, C], bf16, kind="ExternalInput")
    cs2 = nc.dram_tensor("cs2", [P, T], bf16, kind="ExternalInput")  # [cos;cos]
    sn1 = nc.dram_tensor("sn1", [HALF, T], bf16, kind="ExternalInput")  # sin
    maskM = nc.dram_tensor("maskM", [P, P], bf16, kind="ExternalInput")
    y = nc.dram_tensor("y", [B, HPC * D, C], f32, kind="ExternalOutput")

    with tile.TileContext(nc) as tc:
        with tc.tile_pool(name="const", bufs=1) as cp_, \
             tc.tile_pool(name="wo", bufs=1) as wop, \
             tc.tile_pool(name="qkv", bufs=1) as qp, \
             tc.tile_pool(name="ot", bufs=1) as op_, \
             tc.tile_pool(name="ys", bufs=4) as yp, \
             tc.tile_pool(name="small", bufs=2) as sp:

            wq_sb = cp_.tile([P, KO, HPC * D], bf16, tag="wq")
            wk_sb = cp_.tile([P, KO, HPC * D], bf16, tag="wk")
            wv_sb = cp_.tile([P, KO, HPC * D], bf16, tag="wv")
            cs_sb = cp_.tile([P, T], bf16, tag="cs")
            sn_sb = cp_.tile([HALF, T], bf16, tag="sn")
            mask_sb = cp_.tile([P, P], bf16, tag="mask")
            wout_sb = wop.tile([P, KO, C], bf16, tag="wout")

            # startup DMAs: wq first (chunked) so the first chain starts
            # ASAP; the first chunk goes through the gpsimd DGE (its
            # framework preamble ends ~2.5us before the sync engine's).
            nc.gpsimd.dma_start(wq_sb[:, 0:8, :], wq.ap()[:, 0:8, :])
            nc.sync.dma_start(wq_sb[:, 8:16, :], wq.ap()[:, 8:16, :])

            ones_f32 = cp_.tile([P, 1], f32, tag="ones_f32")
            nc.vector.memset(ones_f32[:], 1.0)
            ones_col = cp_.tile([P, 1], bf16, tag="ones_col")
            nc.vector.tensor_copy(ones_col[:], ones_f32[:])
            # act-table warmup: force the Exp table load at t=0 instead of
            # in the middle of the first attention block.
            warm_in = cp_.tile([1, 8], f32, tag="warm_in")
            nc.vector.memset(warm_in[:], 0.0)
            warm_out = cp_.tile([1, 8], f32, tag="warm_out")
            nc.scalar.activation(warm_out[:], warm_in[:], EXP, scale=1.0)

            # persistent attention outputs O^T per (b, local head): [d, t]
            oT = [[op_.tile([P, T], bf16, tag=f"oT{b}{hh}", name=f"oT{b}{hh}")
                   for hh in range(HPC)] for b in range(B)]

            def outproj_chain(b, hh, cpi):
                csl = slice(cpi * TC_, (cpi + 1) * TC_)
                psy = psc_pool[0].tile([P, TC_], f32, tag="y")
                # stationary: oT columns {t : t%16 == j}, strided view
                ovw = oT[b][hh].rearrange("p (u j) -> p j u", j=KO)
                for j in range(KO):
                    nc.tensor.matmul(psy[:], ovw[:, j, :],
                                     wout_sb[:, j, csl],
                                     start=(j == 0), stop=(j == KO - 1))
                ysb = yp.tile([P, TC_], f32, tag="ysb")
                nc.scalar.copy(ysb[:], psy[:])
                nc.sync.dma_start(
                    y.ap()[b, hh * D:(hh + 1) * D, csl], ysb[:])

            psc_pool = [None]

            for b in range(B):
                qT = [qp.tile([P, T], bf16, tag=f"qT{hh}", name=f"qT{b}{hh}")
                      for hh in range(HPC)]
                kT = [qp.tile([P, T], bf16, tag=f"kT{hh}", name=f"kT{b}{hh}")
                      for hh in range(HPC)]
                vt = [qp.tile([P, NSC, D], bf16, tag=f"v{hh}", name=f"v{b}{hh}")
                      for hh in range(HPC)]

                # ---------------- QKV projection + RoPE ----------------
                with tc.tile_pool(name=f"xt{b}", bufs=3) as xp, \
                     tc.tile_pool(name=f"psA{b}", bufs=3, space="PSUM") as psa, \
                     tc.tile_pool(name=f"psV{b}", bufs=2, space="PSUM") as psv_p, \
                     tc.tile_pool(name=f"rope{b}", bufs=3) as rp:

                    def qkmm(xt, w_sb, hh, nm):
                        hsl = slice(hh * D, (hh + 1) * D)
                        ps = psa.tile([P, TQ], f32, tag="acc", name=nm)
                        for ko in range(KO):
                            nc.tensor.matmul(ps[:], w_sb[:, ko, hsl],
                                             xt[:, ko, :],
                                             start=(ko == 0),
                                             stop=(ko == KO - 1))
                        return ps

                    def rope(ps, dst, sl):
                        # tcos = ps * [cos;cos]; tsw pre-swaps halves:
                        # tsw[0:64]=q2*sin, tsw[64:128]=q1*sin so the add/sub
                        # reads align on base partitions.  All elementwise
                        # work on the VE (bf16 operands get 2x mode).
                        cs = cs_sb[:, sl]
                        sn = sn_sb[:, sl]
                        tcos = rp.tile([P, TQ], bf16, tag="tcos")
                        tsw = rp.tile([P, TQ], bf16, tag="tsw")
                        nc.vector.tensor_mul(tcos[:], ps[:], cs)
                        nc.vector.tensor_mul(tsw[0:HALF, :], ps[HALF:P, :], sn)
                        nc.vector.tensor_mul(tsw[HALF:P, :], ps[0:HALF, :], sn)
                        nc.vector.tensor_sub(dst[0:HALF, sl],
                                             tcos[0:HALF, :], tsw[0:HALF, :])
                        nc.vector.tensor_add(dst[HALF:P, sl],
                                             tcos[HALF:P, :], tsw[HALF:P, :])

                    def vchain(xt, ti):
                        for sub in range(TQ // P):
                            psv = psv_p.tile([P, HPC * D], f32, tag="acc")
                            for ko in range(KO):
                                nc.tensor.matmul(
                                    psv[:], xt[:, ko, sub * P:(sub + 1) * P],
                                    wv_sb[:, ko, :],
                                    start=(ko == 0), stop=(ko == KO - 1))
                            tci = ti * (TQ // P) + sub
                            for hh in range(HPC):
                                # Act engine is idle during QKV; it does the
                                # psum->sbuf v copies.
                                nc.scalar.copy(
                                    vt[hh][:, tci, :],
                                    psv[:, hh * D:(hh + 1) * D])

                    xts = {}
                    for ti in range(NT):
                        xts[ti] = xp.tile([P, KO, TQ], bf16, tag="xt",
                                          name=f"xt{b}_{ti}")

                    if b == 0:
                        # Startup is a DMA-bandwidth wall: ~7MB must land in
                        # the first ~30us.  Chunk the first two x tiles so
                        # chains pace behind arriving data, interleave wq/x
                        # chunks in ko-consumption order, and defer ti0's
                        # v-chains until after ti1's q/k so wv is needed
                        # later.
                        for g in range(4):
                            nc.sync.dma_start(
                                xts[0][:, 4 * g:4 * g + 4, :],
                                xTt.ap()[b, 0, :, 4 * g:4 * g + 4, :])
                        ps = qkmm(xts[0], wq_sb, 0, "acc0_q0")
                        nc.sync.dma_start(wk_sb[:], wk.ap())
                        nc.sync.dma_start(cs_sb[:], cs2.ap())
                        nc.sync.dma_start(sn_sb[:], sn1.ap())
                        rope(ps, qT[0], slice(0, TQ))
                        rope(qkmm(xts[0], wq_sb, 1, "acc0_q1"), qT[1],
                             slice(0, TQ))
                        for g in range(4):
                            nc.sync.dma_start(
                                xts[1][:, 4 * g:4 * g + 4, :],
                                xTt.ap()[b, 1, :, 4 * g:4 * g + 4, :])
                        rope(qkmm(xts[0], wk_sb, 0, "acc0_k0"), kT[0],
                             slice(0, TQ))
                        nc.sync.dma_start(wv_sb[:], wv.ap())
                        nc.sync.dma_start(mask_sb[:], maskM.ap())
                        rope(qkmm(xts[0], wk_sb, 1, "acc0_k1"), kT[1],
                             slice(0, TQ))
                        sl1 = slice(TQ, 2 * TQ)
                        rope(qkmm(xts[1], wq_sb, 0, "acc1_q0"), qT[0], sl1)
                        rope(qkmm(xts[1], wq_sb, 1, "acc1_q1"), qT[1], sl1)
                        rope(qkmm(xts[1], wk_sb, 0, "acc1_k0"), kT[0], sl1)
                        rope(qkmm(xts[1], wk_sb, 1, "acc1_k1"), kT[1], sl1)
                        vchain(xts[1], 1)
                        vchain(xts[0], 0)
                        rest = range(2, NT)
                    else:
                        rest = range(NT)

                    for ti in rest:
                        sl = slice(ti * TQ, (ti + 1) * TQ)
                        xt = xts[ti]
                        nc.sync.dma_start(xt[:], xTt.ap()[b, ti])
                        for hh in range(HPC):
                            rope(qkmm(xt, wq_sb, hh, f"a{ti}q{hh}"),
                                 qT[hh], sl)
                            rope(qkmm(xt, wk_sb, hh, f"a{ti}k{hh}"),
                                 kT[hh], sl)
                        vchain(xt, ti)

                # ------------- attention (S^T layout) + interleaved -----
                # ------------- out-projection of the previous head ------
                # s-chunks are processed in PAIRS sharing one 2-bank PSUM
                # tile and a single exp instruction, so the Act engine
                # (1024 cols + one fixed overhead) runs faster than the
                # PE's 6 matmuls per pair and never paces the pipeline.
                with tc.tile_pool(name=f"psBsc{b}", bufs=2, space="PSUM") as pssc, \
                     tc.tile_pool(name=f"psBo{b}", bufs=2, space="PSUM") as pso, \
                     tc.tile_pool(name=f"psBsum{b}", bufs=2, space="PSUM") as pssum, \
                     tc.tile_pool(name=f"pt{b}", bufs=6) as ptp:
                    nwo = 0   # wout prefetch cursor (b == 0 only)
                    # Deferred-issue queue: each t-tile's normalize is
                    # issued inside the NEXT tile's block loop (ps_o and
                    # ps_sum have bufs=2, so it must be issued before the
                    # slot cycles) -- the PE/Act pipeline never waits on it.
                    norm_q = []

                    for hh in range(HPC):
                        for ta in range(NTA):
                            spt = TA // P
                            tsl = slice(ta * TA, (ta + 1) * TA)
                            ps_o = pso.tile([P, TA], f32, tag="o")
                            ps_sum = pssum.tile([1, TA], f32, tag="sum")
                            nblk = (ta + 1) * spt
                            pend = []

                            def flush(last):
                                # same-accumulation-group matmuls must be
                                # adjacent: a LDWEIGHTS that follows an
                                # accumulating matmul whose group is being
                                # suspended stalls ~95ns on hw (after a
                                # STOPPED group it is free).  Flush up to
                                # FOUR pairs at once, all o-matmuls in one
                                # burst then all sum-matmuls, so only two
                                # group suspensions happen per flush.
                                take, pend[:] = pend[:4], pend[4:]
                                mms = [(pt_, k, s_, w_)
                                       for pt_, sws in take
                                       for k, (s_, w_) in enumerate(sws)]
                                for i, (pt_, k, s_, w_) in enumerate(mms):
                                    nc.tensor.matmul(ps_o[:, w_],
                                                     vt[hh][:, s_, :],
                                                     pt_[:, k, w_],
                                                     start=(s_ == 0),
                                                     stop=(last and
                                                           i == len(mms) - 1))
                                for i, (pt_, k, s_, w_) in enumerate(mms):
                                    nc.tensor.matmul(
                                        ps_sum[:, w_],
                                        ones_col[:], pt_[:, k, w_],
                                        start=(s_ == 0),
                                        stop=(last and i == len(mms) - 1))

                            for pi in range(nblk // 2):
                                ps_sc = pssc.tile([P, 2, TA], f32, tag="sc")
                                pt = ptp.tile([P, 2, TA], bf16, tag="pt")
                                sws = []
                                for k in range(2):
                                    s = 2 * pi + k
                                    diag = s >= ta * spt
                                    t_lo = (s - ta * spt) * P if diag else 0
                                    w = slice(t_lo, TA)
                                    qsl = slice(ta * TA + t_lo,
                                                (ta + 1) * TA)
                                    nc.tensor.matmul(
                                        ps_sc[:, k, w],
                                        kT[hh][:, s * P:(s + 1) * P],
                                        qT[hh][:, qsl],
                                        start=True, stop=True)
                                    sws.append((s, w))
                                # one exp for both chunks; cols outside a
                                # diag chunk's window hold stale psum ->
                                # garbage pt that no matmul reads
                                nc.scalar.activation(pt[:, :, :],
                                                     ps_sc[:, :, :],
                                                     EXP, scale=SCALE)
                                for k, (s, w) in enumerate(sws):
                                    if s >= ta * spt:  # mask the triangle
                                        t_lo = (s - ta * spt) * P
                                        nc.vector.tensor_mul(
                                            pt[:, k, t_lo:t_lo + P],
                                            pt[:, k, t_lo:t_lo + P],
                                            mask_sb[:])
                                pend.append((pt, sws))
                                if len(pend) > 4:
                                    flush(False)
                                if pi == 0 and norm_q:
                                    norm_q.pop(0)()
                            flush(True)

                            # normalization, deferred: recip on VE,
                            # partition-broadcast on gpsimd, flat multiply
                            # on VE -- issued inside the next tile's block
                            # loop so the PE/Act pipeline never waits.
                            def normalize(ps_o=ps_o, ps_sum=ps_sum,
                                          hh=hh, tsl=tsl):
                                recf = sp.tile([1, TA], f32, tag="recf")
                                nc.vector.reciprocal_approx_fast(
                                    recf[:], ps_sum[:])
                                recb = sp.tile([1, TA], bf16, tag="recb")
                                nc.vector.tensor_copy(recb[:], recf[:])
                                bcb = sp.tile([P, TA], bf16, tag="bcb")
                                nc.gpsimd.partition_broadcast(bcb[:],
                                                              recb[:],
                                                              channels=P)
                                nc.vector.tensor_mul(oT[b][hh][:, tsl],
                                                     ps_o[:], bcb[:])
                            norm_q.append(normalize)

                            if b == 0 and hh == 0:
                                # prefetch all of wout during head 0's
                                # attention (the first out-proj chain needs
                                # every j block)
                                for _ in range(4):
                                    nc.sync.dma_start(
                                        wout_sb[:, nwo, :], woutT.ap()[nwo])
                                    nwo += 1

                    for t_ in norm_q:
                        t_()

                # ---------------- output projection (pure PE) ----------
                with tc.tile_pool(name=f"psC{b}", bufs=2, space="PSUM") as psc:
                    psc_pool[0] = psc
                    for hh in range(HPC):
                        for cpi in range(NCP):
                            outproj_chain(b, hh, cpi)

    nc.compile()
    return nc


_NC = None


def _get_nc():
    global _NC
    if _NC is None:
        _NC = _build()
    return _NC


def _host_tables():
    pos = np.arange(T, dtype=np.float32)[:, None]
    div = np.exp(np.arange(0, 2 * HALF, 2, dtype=np.float32)
                 * np.float32(-math.log(ROPE_BASE) / (2 * HALF)))
    ang = pos * div[None, :]
    cosv = np.cos(ang).astype(np.float32)   # [T, HALF]
    sinv = np.sin(ang).astype(np.float32)
    cosT = np.ascontiguousarray(cosv.T)     # [HALF, T]
    sinT = np.ascontiguousarray(sinv.T)
    cs2 = np.ascontiguousarray(
        np.concatenate([cosT, cosT], axis=0)).astype(ml_dtypes.bfloat16)
    sn1 = np.ascontiguousarray(sinT).astype(ml_dtypes.bfloat16)
    # triangle mask M[s, w] = 1 iff s <= w
    ww = np.arange(P)[None, :]
    ss = np.arange(P)[:, None]
    maskM = (ss <= ww).astype(ml_dtypes.bfloat16)
    return cs2, sn1, maskM


def _make_in_maps(x, Wqkv, Wout):
    x = np.asarray(x, dtype=np.float32)
    Wqkv = np.asarray(Wqkv, dtype=np.float32)
    Wout = np.asarray(Wout, dtype=np.float32)
    assert x.shape == (B, T, C) and Wqkv.shape == (C, 3 * C) \
        and Wout.shape == (C, C)

    cs2, sn1, maskM = _host_tables()
    # xTt[b, ti, p, ko, u] = x[b, ti*TQ+u, ko*128+p]
    xTt = np.ascontiguousarray(
        x.reshape(B, NT, TQ, KO, P).transpose(0, 1, 4, 3, 2)
    ).astype(ml_dtypes.bfloat16)
    woutT = np.ascontiguousarray(
        Wout.reshape(KO, P, C)).astype(ml_dtypes.bfloat16)

    in_maps = []
    for core in range(NCORES):
        h0 = core * HPC
        cols = slice(h0 * D, (h0 + HPC) * D)
        ws = []
        for part in range(3):
            w = Wqkv[:, part * C:(part + 1) * C][:, cols]  # [C, HPC*D]
            ws.append(np.ascontiguousarray(
                w.reshape(KO, P, HPC * D).transpose(1, 0, 2)
            ).astype(ml_dtypes.bfloat16))
        in_maps.append({
            "xTt": xTt,
            "wq": ws[0], "wk": ws[1], "wv": ws[2],
            "woutT": woutT,
            "cs2": cs2, "sn1": sn1, "maskM": maskM,
        })
    return in_maps


def _run(x, Wqkv, Wout, trace=False):
    nc = _get_nc()
    in_maps = _make_in_maps(x, Wqkv, Wout)
    res = run_bass_kernel_spmd(nc, in_maps, core_ids=list(range(NCORES)),
                               trace=trace)
    out = np.empty((B, T, C), dtype=np.float32)
    for core in range(NCORES):
        out[:, core * HPC * D:(core + 1) * HPC * D, :] = \
            res.results[core]["y"]
    return out, res


def kernel(x, Wqkv, Wout):
    out, _ = _run(x, Wqkv, Wout)
    return out


# revision 39
# speedup vs baseline: 1.0213x; 1.0213x over previous
"""Trainium2 Bass kernel for nn_MultiHeadAttention_63015760167496.

Computation (see reference): qkv = x @ Wqkv; RoPE on q,k; causal softmax
attention per head; out = einsum('bhts,bshd->bhtd', probs, v);
out.reshape(B,T,C) @ Wout  -- the reshape is a *head-major* flatten of
[B,H,T,D] into [B,T,C], so final-output row r = h*128 + t//16 depends only
on head h.  Sharding: head-parallel over 8 cores (2 heads/core); every core
computes its two heads end-to-end and produces final-output rows
[256*i, 256*i+256).  Host concatenates -- no collectives.

All on-device data is bf16 (PSUM accumulation f32), which halves DMA/SBUF
vs f32r at the same 1 cycle/row PE rate.  Attention runs in S^T layout
([s,t]): softmax denominator via a ones-column matmul (partition reduction
on the PE), normalization broadcast via gpsimd.partition_broadcast and a
flat VE multiply, both issued *deferred* (inside the next t-tile's block
loop) so they never gate the PE.  The attention inner loop is
software-pipelined: score blocks are processed in pairs sharing one
two-bank PSUM tile and a single exp instruction, and the PV/sum matmuls of
the previous two pairs are flushed in same-accumulation-group bursts (a
LDWEIGHTS after an accumulating matmul whose group is suspended stalls
~95ns on hw).  O^T is stored flat [d, t]; the out-projection reads it
through a strided LDWEIGHTS view.  Wout is prefetched into SBUF during
attention b=0 so the out-projection phases are pure PE.
"""

import math
import sys

for _p in ("/opt/trn_rl_repo", "/root/.axon_site/_ro/trn_rl_repo"):
    if _p not in sys.path:
        sys.path.insert(0, _p)

import numpy as np
import ml_dtypes

import concourse.bass as bass
import concourse.mybir as mybir
import concourse.tile as tile
from concourse import bacc
from concourse.bass_utils import run_bass_kernel_spmd

B, T, C = 2, 2048, 2048
H = 16            # heads total
D = C // H        # 128 head dim
HALF = D // 2     # 64
P = 128
KO = C // P       # 16 contraction chunks
NCORES = 8
HPC = H // NCORES  # 2 heads per core
TQ = 512          # t-tile for qkv projection
NT = T // TQ      # 4
TA = 512          # t-tile for attention
NTA = T // TA     # 4
NSC = T // P      # 16 s-chunks
ROPE_BASE = 10000.0
SCALE = 1.0 / math.sqrt(D)
TC_ = 512         # col-tile for out projection
NCP = C // TC_    # 4
LOOKAHEAD = 2     # attention software-pipeline depth

f32 = mybir.dt.float32
bf16 = mybir.dt.bfloat16
EXP = mybir.ActivationFunctionType.Exp


def _build():
    nc = bacc.Bacc("TRN2", target_bir_lowering=False, debug=False,
                   num_devices=NCORES)

    # host-pre-tiled x^T: xTt[b, ti, p, ko, u] = x[b, ti*TQ+u, ko*128+p]
    xTt = nc.dram_tensor("xTt", [B, NT, P, KO, TQ], bf16, kind="ExternalInput")
    # host-pre-chunked weights: w[p, ko, m] = W[ko*128+p, m]
    wq = nc.dram_tensor("wq", [P, KO, HPC * D], bf16, kind="ExternalInput")
    wk = nc.dram_tensor("wk", [P, KO, HPC * D], bf16, kind="ExternalInput")
    wv = nc.dram_tensor("wv", [P, KO, HPC * D], bf16, kind="ExternalInput")
    # woutT[j, p, c] = Wout[j*128+p, c]
    woutT = nc.dram_tensor("woutT", [KO, P, C], bf16, kind="ExternalInput")
    cs2 = nc.dram_tensor("cs2", [P, T], bf16, kind="ExternalInput")  # [cos;cos]
    sn1 = nc.dram_tensor("sn1", [HALF, T], bf16, kind="ExternalInput")  # sin
    maskM = nc.dram_tensor("maskM", [P, P], bf16, kind="ExternalInput")
    y = nc.dram_tensor("y", [B, HPC * D, C], f32, kind="ExternalOutput")

    with tile.TileContext(nc) as tc:
        with tc.tile_pool(name="const", bufs=1) as cp_, \
             tc.tile_pool(name="wo", bufs=1) as wop, \
             tc.tile_pool(name="qkv", bufs=1) as qp, \
             tc.tile_pool(name="ot", bufs=1) as op_, \
             tc.tile_pool(name="ys", bufs=4) as yp, \
             tc.tile_pool(name="small", bufs=2) as sp:

            wq_sb = cp_.tile([P, KO, HPC * D], bf16, tag="wq")
            wk_sb = cp_.tile([P, KO, HPC * D], bf16, tag="wk")
            wv_sb = cp_.tile([P, KO, HPC * D], bf16, tag="wv")
            cs_sb = cp_.tile([P, T], bf16, tag="cs")
            sn_sb = cp_.tile([HALF, T], bf16, tag="sn")
            mask_sb = cp_.tile([P, P], bf16, tag="mask")
            wout_sb = wop.tile([P, KO, C], bf16, tag="wout")

            # startup DMAs: wq first (chunked) so the first chain starts
            # ASAP; the first chunk goes through the gpsimd DGE (its
            # framework preamble ends ~2.5us before the sync engine's).
            nc.gpsimd.dma_start(wq_sb[:, 0:8, :], wq.ap()[:, 0:8, :])
            nc.sync.dma_start(wq_sb[:, 8:16, :], wq.ap()[:, 8:16, :])

            ones_f32 = cp_.tile([P, 1], f32, tag="ones_f32")
            nc.vector.memset(ones_f32[:], 1.0)
            ones_col = cp_.tile([P, 1], bf16, tag="ones_col")
            nc.vector.tensor_copy(ones_col[:], ones_f32[:])
            # act-table warmup: force the Exp table load at t=0 instead of
            # in the middle of the first attention block.
            warm_in = cp_.tile([1, 8], f32, tag="warm_in")
            nc.vector.memset(warm_in[:], 0.0)
            warm_out = cp_.tile([1, 8], f32, tag="warm_out")
            nc.scalar.activation(warm_out[:], warm_in[:], EXP, scale=1.0)

            # persistent attention outputs O^T per (b, local head): [d, t]
            oT = [[op_.tile([P, T], bf16, tag=f"oT{b}{hh}", name=f"oT{b}{hh}")
                   for hh in range(HPC)] for b in range(B)]

            def outproj_chain(b, hh, cpi):
                csl = slice(cpi * TC_, (cpi + 1) * TC_)
                psy = psc_pool[0].tile([P, TC_], f32, tag="y")
                # stationary: oT columns {t : t%16 == j}, strided view
                ovw = oT[b][hh].rearrange("p (u j) -> p j u", j=KO)
                for j in range(KO):
                    nc.tensor.matmul(psy[:], ovw[:, j, :],
                                     wout_sb[:, j, csl],
                                     start=(j == 0), stop=(j == KO - 1))
                ysb = yp.tile([P, TC_], f32, tag="ysb")
                nc.scalar.copy(ysb[:], psy[:])
                nc.sync.dma_start(
                    y.ap()[b, hh * D:(hh + 1) * D, csl], ysb[:])

            psc_pool = [None]

            for b in range(B):
                qT = [qp.tile([P, T], bf16, tag=f"qT{hh}", name=f"qT{b}{hh}")
                      for hh in range(HPC)]
                kT = [qp.tile([P, T], bf16, tag=f"kT{hh}", name=f"kT{b}{hh}")
                      for hh in range(HPC)]
                vt = [qp.tile([P, NSC, D], bf16, tag=f"v{hh}", name=f"v{b}{hh}")
                      for hh in range(HPC)]

                # ---------------- QKV projection + RoPE ----------------
                with tc.tile_pool(name=f"xt{b}", bufs=3) as xp, \
                     tc.tile_pool(name=f"psA{b}", bufs=3, space="PSUM") as psa, \
                     tc.tile_pool(name=f"psV{b}", bufs=2, space="PSUM") as psv_p, \
                     tc.tile_pool(name=f"rope{b}", bufs=3) as rp:

                    def qkmm(xt, w_sb, hh, nm):
                        hsl = slice(hh * D, (hh + 1) * D)
                        ps = psa.tile([P, TQ], f32, tag="acc", name=nm)
                        for ko in range(KO):
                            nc.tensor.matmul(ps[:], w_sb[:, ko, hsl],
                                             xt[:, ko, :],
                                             start=(ko == 0),
                                             stop=(ko == KO - 1))
                        return ps

                    def rope(ps, dst, sl):
                        # tcos = ps * [cos;cos]; tsw pre-swaps halves:
                        # tsw[0:64]=q2*sin, tsw[64:128]=q1*sin so the add/sub
                        # reads align on base partitions.  All elementwise
                        # work on the VE (bf16 operands get 2x mode).
                        cs = cs_sb[:, sl]
                        sn = sn_sb[:, sl]
                        tcos = rp.tile([P, TQ], bf16, tag="tcos")
                        tsw = rp.tile([P, TQ], bf16, tag="tsw")
                        nc.vector.tensor_mul(tcos[:], ps[:], cs)
                        nc.vector.tensor_mul(tsw[0:HALF, :], ps[HALF:P, :], sn)
                        nc.vector.tensor_mul(tsw[HALF:P, :], ps[0:HALF, :], sn)
                        nc.vector.tensor_sub(dst[0:HALF, sl],
                                             tcos[0:HALF, :], tsw[0:HALF, :])
                        nc.vector.tensor_add(dst[HALF:P, sl],
                                             tcos[HALF:P, :], tsw[HALF:P, :])

                    def vchain(xt, ti):
                        for sub in range(TQ // P):
                            psv = psv_p.tile([P, HPC * D], f32, tag="acc")
                            for ko in range(KO):
                                nc.tensor.matmul(
                                    psv[:], xt[:, ko, sub * P:(sub + 1) * P],
                                    wv_sb[:, ko, :],
                                    start=(ko == 0), stop=(ko == KO - 1))
                            tci = ti * (TQ // P) + sub
                            for hh in range(HPC):
                                # Act engine is idle during QKV; it does the
                                # psum->sbuf v copies.
                                nc.scalar.copy(
                                    vt[hh][:, tci, :],
                                    psv[:, hh * D:(hh + 1) * D])

                    xts = {}
                    for ti in range(NT):
                        xts[ti] = xp.tile([P, KO, TQ], bf16, tag="xt",
                                          name=f"xt{b}_{ti}")

                    if b == 0:
                        # Startup is a DMA-bandwidth wall: ~7MB must land in
                        # the first ~30us.  Chunk the first two x tiles so
                        # chains pace behind arriving data, interleave wq/x
                        # chunks in ko-consumption order, and defer ti0's
                        # v-chains until after ti1's q/k so wv is needed
                        # later.
                        for g in range(4):
                            nc.sync.dma_start(
                                xts[0][:, 4 * g:4 * g + 4, :],
                                xTt.ap()[b, 0, :, 4 * g:4 * g + 4, :])
                        ps = qkmm(xts[0], wq_sb, 0, "acc0_q0")
                        nc.sync.dma_start(wk_sb[:], wk.ap())
                        nc.sync.dma_start(cs_sb[:], cs2.ap())
                        nc.sync.dma_start(sn_sb[:], sn1.ap())
                        rope(ps, qT[0], slice(0, TQ))
                        rope(qkmm(xts[0], wq_sb, 1, "acc0_q1"), qT[1],
                             slice(0, TQ))
                        for g in range(4):
                            nc.sync.dma_start(
                                xts[1][:, 4 * g:4 * g + 4, :],
                                xTt.ap()[b, 1, :, 4 * g:4 * g + 4, :])
                        rope(qkmm(xts[0], wk_sb, 0, "acc0_k0"), kT[0],
                             slice(0, TQ))
                        nc.sync.dma_start(wv_sb[:], wv.ap())
                        nc.sync.dma_start(mask_sb[:], maskM.ap())
                        rope(qkmm(xts[0], wk_sb, 1, "acc0_k1"), kT[1],
                             slice(0, TQ))
                        sl1 = slice(TQ, 2 * TQ)
                        rope(qkmm(xts[1], wq_sb, 0, "acc1_q0"), qT[0], sl1)
                        rope(qkmm(xts[1], wq_sb, 1, "acc1_q1"), qT[1], sl1)
                        rope(qkmm(xts[1], wk_sb, 0, "acc1_k0"), kT[0], sl1)
                        rope(qkmm(xts[1], wk_sb, 1, "acc1_k1"), kT[1], sl1)
                        vchain(xts[1], 1)
                        vchain(xts[0], 0)
                        rest = range(2, NT)
                    else:
                        rest = range(NT)

                    for ti in rest:
                        sl = slice(ti * TQ, (ti + 1) * TQ)
                        xt = xts[ti]
                        nc.sync.dma_start(xt[:], xTt.ap()[b, ti])
                        for hh in range(HPC):
                            rope(qkmm(xt, wq_sb, hh, f"a{ti}q{hh}"),
                                 qT[hh], sl)
                            rope(qkmm(xt, wk_sb, hh, f"a{ti}k{hh}"),
                                 kT[hh], sl)
                        vchain(xt, ti)

                # ------------- attention (S^T layout) + interleaved -----
                # ------------- out-projection of the previous head ------
                # s-chunks are processed in PAIRS sharing one 2-bank PSUM
                # tile and a single exp instruction, so the Act engine
                # (1024 cols + one fixed overhead) runs faster than the
                # PE's 6 matmuls per pair and never paces the pipeline.
                with tc.tile_pool(name=f"psBsc{b}", bufs=2, space="PSUM") as pssc, \
                     tc.tile_pool(name=f"psBo{b}", bufs=2, space="PSUM") as pso, \
                     tc.tile_pool(name=f"psBsum{b}", bufs=2, space="PSUM") as pssum, \
                     tc.tile_pool(name=f"pt{b}", bufs=4) as ptp:
                    nwo = 0   # wout prefetch cursor (b == 0 only)
                    # Deferred-issue queue: each t-tile's normalize is
                    # issued inside the NEXT tile's block loop (ps_o and
                    # ps_sum have bufs=2, so it must be issued before the
                    # slot cycles) -- the PE/Act pipeline never waits on it.
                    norm_q = []

                    for hh in range(HPC):
                        for ta in range(NTA):
                            spt = TA // P
                            tsl = slice(ta * TA, (ta + 1) * TA)
                            ps_o = pso.tile([P, TA], f32, tag="o")
                            ps_sum = pssum.tile([1, TA], f32, tag="sum")
                            nblk = (ta + 1) * spt
                            pend = []

                            def flush(last):
                                # same-accumulation-group matmuls must be
                                # adjacent: a LDWEIGHTS that follows an
                                # accumulating matmul whose group is being
                                # suspended stalls ~95ns on hw (after a
                                # STOPPED group it is free).  Flush up to
                                # TWO pairs at once, all o-matmuls in one
                                # burst then all sum-matmuls, so only two
                                # group suspensions happen per flush.
                                take, pend[:] = pend[:2], pend[2:]
                                mms = [(pt_, k, s_, w_)
                                       for pt_, sws in take
                                       for k, (s_, w_) in enumerate(sws)]
                                for i, (pt_, k, s_, w_) in enumerate(mms):
                                    nc.tensor.matmul(ps_o[:, w_],
                                                     vt[hh][:, s_, :],
                                                     pt_[:, k, w_],
                                                     start=(s_ == 0),
                                                     stop=(last and
                                                           i == len(mms) - 1))
                                for i, (pt_, k, s_, w_) in enumerate(mms):
                                    nc.tensor.matmul(
                                        ps_sum[:, w_],
                                        ones_col[:], pt_[:, k, w_],
                                        start=(s_ == 0),
                                        stop=(last and i == len(mms) - 1))

                            for pi in range(nblk // 2):
                                ps_sc = pssc.tile([P, 2, TA], f32, tag="sc")
                                pt = ptp.tile([P, 2, TA], bf16, tag="pt")
                                sws = []
                                for k in range(2):
                                    s = 2 * pi + k
                                    diag = s >= ta * spt
                                    t_lo = (s - ta * spt) * P if diag else 0
                                    w = slice(t_lo, TA)
                                    qsl = slice(ta * TA + t_lo,
                                                (ta + 1) * TA)
                                    nc.tensor.matmul(
                                        ps_sc[:, k, w],
                                        kT[hh][:, s * P:(s + 1) * P],
                                        qT[hh][:, qsl],
                                        start=True, stop=True)
                                    sws.append((s, w))
                                # one exp for both chunks; cols outside a
                                # diag chunk's window hold stale psum ->
                                # garbage pt that no matmul reads
                                nc.scalar.activation(pt[:, :, :],
                                                     ps_sc[:, :, :],
                                                     EXP, scale=SCALE)
                                for k, (s, w) in enumerate(sws):
                                    if s >= ta * spt:  # mask the triangle
                                        t_lo = (s - ta * spt) * P
                                        nc.vector.tensor_mul(
                                            pt[:, k, t_lo:t_lo + P],
                                            pt[:, k, t_lo:t_lo + P],
                                            mask_sb[:])
                                pend.append((pt, sws))
                                if len(pend) > 2:
                                    flush(False)
                                if pi == 0 and norm_q:
                                    norm_q.pop(0)()
                            flush(True)

                            # normalization, deferred: recip on VE,
                            # partition-broadcast on gpsimd, flat multiply
                            # on VE -- issued inside the next tile's block
                            # loop so the PE/Act pipeline never waits.
                            def normalize(ps_o=ps_o, ps_sum=ps_sum,
                                          hh=hh, tsl=tsl):
                                recf = sp.tile([1, TA], f32, tag="recf")
                                nc.vector.reciprocal_approx_fast(
                                    recf[:], ps_sum[:])
                                recb = sp.tile([1, TA], bf16, tag="recb")
                                nc.vector.tensor_copy(recb[:], recf[:])
                                bcb = sp.tile([P, TA], bf16, tag="bcb")
                                nc.gpsimd.partition_broadcast(bcb[:],
                                                              recb[:],
                                                              channels=P)
                                nc.vector.tensor_mul(oT[b][hh][:, tsl],
                                                     ps_o[:], bcb[:])
                            norm_q.append(normalize)

                            if b == 0 and hh == 0:
                                # prefetch all of wout during head 0's
                                # attention (the first out-proj chain needs
                                # every j block)
                                for _ in range(4):
                                    nc.sync.dma_start(
                                        wout_sb[:, nwo, :], woutT.ap()[nwo])
                                    nwo += 1

                    for t_ in norm_q:
                        t_()

                # ---------------- output projection (pure PE) ----------
                with tc.tile_pool(name=f"psC{b}", bufs=2, space="PSUM") as psc:
                    psc_pool[0] = psc
                    for hh in range(HPC):
                        for cpi in range(NCP):
                            outproj_chain(b, hh, cpi)

    nc.compile()
    return nc


_NC = None


def _get_nc():
    global _NC
    if _NC is None:
        _NC = _build()
    return _NC


def _host_tables():
    pos = np.arange(T, dtype=np.float32)[:, None]
    div = np.exp(np.arange(0, 2 * HALF, 2, dtype=np.float32)
                 * np.float32(-math.log(ROPE_BASE) / (2 * HALF)))
    ang = pos * div[None, :]
    cosv = np.cos(ang).astype(np.float32)   # [T, HALF]
    sinv = np.sin(ang).astype(np.float32)
    cosT = np.ascontiguousarray(cosv.T)     # [HALF, T]
    sinT = np.ascontiguousarray(sinv.T)
    cs2 = np.ascontiguousarray(
        np.concatenate([cosT, cosT], axis=0)).astype(ml_dtypes.bfloat16)
    sn1 = np.ascontiguousarray(sinT).astype(ml_dtypes.bfloat16)
    # triangle mask M[s, w] = 1 iff s <= w
    ww = np.arange(P)[None, :]
    ss = np.arange(P)[:, None]
    maskM = (ss <= ww).astype(ml_dtypes.bfloat16)
    return cs2, sn1, maskM


def _make_in_maps(x, Wqkv, Wout):
    x = np.asarray(x, dtype=np.float32)
    Wqkv = np.asarray(Wqkv, dtype=np.float32)
    Wout = np.asarray(Wout, dtype=np.float32)
    assert x.shape == (B, T, C) and Wqkv.shape == (C, 3 * C) \
        and Wout.shape == (C, C)

    cs2, sn1, maskM = _host_tables()
    # xTt[b, ti, p, ko, u] = x[b, ti*TQ+u, ko*128+p]
    xTt = np.ascontiguousarray(
        x.reshape(B, NT, TQ, KO, P).transpose(0, 1, 4, 3, 2)
    ).astype(ml_dtypes.bfloat16)
    woutT = np.ascontiguousarray(
        Wout.reshape(KO, P, C)).astype(ml_dtypes.bfloat16)

    in_maps = []
    for core in range(NCORES):
        h0 = core * HPC
        cols = slice(h0 * D, (h0 + HPC) * D)
        ws = []
        for part in range(3):
            w = Wqkv[:, part * C:(part + 1) * C][:, cols]  # [C, HPC*D]
            ws.append(np.ascontiguousarray(
                w.reshape(KO, P, HPC * D).transpose(1, 0, 2)
            ).astype(ml_dtypes.bfloat16))
        in_maps.append({
            "xTt": xTt,
            "wq": ws[0], "wk": ws[1], "wv": ws[2],
            "woutT": woutT,
            "cs2": cs2, "sn1": sn1, "maskM": maskM,
        })
    return in_maps


def _run(x, Wqkv, Wout, trace=False):
    nc = _get_nc()
    in_maps = _make_in_maps(x, Wqkv, Wout)
    res = run_bass_kernel_spmd(nc, in_maps, core_ids=list(range(NCORES)),
                               trace=trace)
    out = np.empty((B, T, C), dtype=np.float32)
    for core in range(NCORES):
        out[:, core * HPC * D:(core + 1) * HPC * D, :] = \
            res.results[core]["y"]
    return out, res


def kernel(x, Wqkv, Wout):
    out, _ = _run(x, Wqkv, Wout)
    return out


# revision 42
# speedup vs baseline: 1.0216x; 1.0003x over previous
"""Trainium2 Bass kernel for nn_MultiHeadAttention_63015760167496.

Computation (see reference): qkv = x @ Wqkv; RoPE on q,k; causal softmax
attention per head; out = einsum('bhts,bshd->bhtd', probs, v);
out.reshape(B,T,C) @ Wout  -- the reshape is a *head-major* flatten of
[B,H,T,D] into [B,T,C], so final-output row r = h*128 + t//16 depends only
on head h.  Sharding: head-parallel over 8 cores (2 heads/core); every core
computes its two heads end-to-end and produces final-output rows
[256*i, 256*i+256).  Host concatenates -- no collectives.

All on-device data is bf16 (PSUM accumulation f32), which halves DMA/SBUF
vs f32r at the same 1 cycle/row PE rate.  Attention runs in S^T layout
([s,t]): softmax denominator via a ones-column matmul (partition reduction
on the PE), normalization broadcast via gpsimd.partition_broadcast and a
flat VE multiply, both issued *deferred* (inside the next t-tile's block
loop) so they never gate the PE.  The attention inner loop is
software-pipelined: score blocks are processed in pairs sharing one
two-bank PSUM tile and a single exp instruction, and the PV/sum matmuls of
the previous two pairs are flushed in same-accumulation-group bursts (a
LDWEIGHTS after an accumulating matmul whose group is suspended stalls
~95ns on hw).  O^T is stored flat [d, t]; the out-projection reads it
through a strided LDWEIGHTS view.  Wout is prefetched into SBUF during
attention b=0 so the out-projection phases are pure PE.
"""

import math
import sys

for _p in ("/opt/trn_rl_repo", "/root/.axon_site/_ro/trn_rl_repo"):
    if _p not in sys.path:
        sys.path.insert(0, _p)

import numpy as np
import ml_dtypes

import concourse.bass as bass
import concourse.mybir as mybir
import concourse.tile as tile
from concourse import bacc
from concourse.bass_utils import run_bass_kernel_spmd

B, T, C = 2, 2048, 2048
H = 16            # heads total
D = C // H        # 128 head dim
HALF = D // 2     # 64
P = 128
KO = C // P       # 16 contraction chunks
NCORES = 8
HPC = H // NCORES  # 2 heads per core
TQ = 512          # t-tile for qkv projection
NT = T // TQ      # 4
TA = 512          # t-tile for attention
NTA = T // TA     # 4
NSC = T // P      # 16 s-chunks
ROPE_BASE = 10000.0
SCALE = 1.0 / math.sqrt(D)
TC_ = 512         # col-tile for out projection
NCP = C // TC_    # 4
LOOKAHEAD = 2     # attention software-pipeline depth

f32 = mybir.dt.float32
bf16 = mybir.dt.bfloat16
EXP = mybir.ActivationFunctionType.Exp


def _build():
    nc = bacc.Bacc("TRN2", target_bir_lowering=False, debug=False,
                   num_devices=NCORES)

    # host-pre-tiled x^T: xTt[b, ti, p, ko, u] = x[b, ti*TQ+u, ko*128+p]
    xTt = nc.dram_tensor("xTt", [B, NT, P, KO, TQ], bf16, kind="ExternalInput")
    # host-pre-chunked weights: w[p, ko, m] = W[ko*128+p, m]
    wq = nc.dram_tensor("wq", [P, KO, HPC * D], bf16, kind="ExternalInput")
    wk = nc.dram_tensor("wk", [P, KO, HPC * D], bf16, kind="ExternalInput")
    wv = nc.dram_tensor("wv", [P, KO, HPC * D], bf16, kind="ExternalInput")
    # woutT[j, p, c] = Wout[j*128+p, c]
    woutT = nc.dram_tensor("woutT", [KO, P, C], bf16, kind="ExternalInput")
    cs2 = nc.dram_tensor("cs2", [P, T], bf16, kind="ExternalInput")  # [cos;cos]
    sn1 = nc.dram_tensor("sn1", [HALF, T], bf16, kind="ExternalInput")  # sin
    maskM = nc.dram_tensor("maskM", [P, P], bf16, kind="ExternalInput")
    y = nc.dram_tensor("y", [B, HPC * D, C], f32, kind="ExternalOutput")

    with tile.TileContext(nc) as tc:
        with tc.tile_pool(name="const", bufs=1) as cp_, \
             tc.tile_pool(name="wo", bufs=1) as wop, \
             tc.tile_pool(name="qkv", bufs=1) as qp, \
             tc.tile_pool(name="ot", bufs=1) as op_, \
             tc.tile_pool(name="ys", bufs=4) as yp, \
             tc.tile_pool(name="small", bufs=2) as sp:

            wq_sb = cp_.tile([P, KO, HPC * D], bf16, tag="wq")
            wk_sb = cp_.tile([P, KO, HPC * D], bf16, tag="wk")
            wv_sb = cp_.tile([P, KO, HPC * D], bf16, tag="wv")
            cs_sb = cp_.tile([P, T], bf16, tag="cs")
            sn_sb = cp_.tile([HALF, T], bf16, tag="sn")
            mask_sb = cp_.tile([P, P], bf16, tag="mask")
            wout_sb = wop.tile([P, KO, C], bf16, tag="wout")

            # startup DMAs: wq first (chunked) so the first chain starts
            # ASAP; the first chunk goes through the gpsimd DGE (its
            # framework preamble ends ~2.5us before the sync engine's).
            nc.gpsimd.dma_start(wq_sb[:, 0:8, :], wq.ap()[:, 0:8, :])
            nc.sync.dma_start(wq_sb[:, 8:16, :], wq.ap()[:, 8:16, :])

            ones_f32 = cp_.tile([P, 1], f32, tag="ones_f32")
            nc.vector.memset(ones_f32[:], 1.0)
            ones_col = cp_.tile([P, 1], bf16, tag="ones_col")
            nc.vector.tensor_copy(ones_col[:], ones_f32[:])
            # act-table warmup: force the Exp table load at t=0 instead of
            # in the middle of the first attention block.
            warm_in = cp_.tile([1, 8], f32, tag="warm_in")
            nc.vector.memset(warm_in[:], 0.0)
            warm_out = cp_.tile([1, 8], f32, tag="warm_out")
            nc.scalar.activation(warm_out[:], warm_in[:], EXP, scale=1.0)

            # persistent attention outputs O^T per (b, local head): [d, t]
            oT = [[op_.tile([P, T], bf16, tag=f"oT{b}{hh}", name=f"oT{b}{hh}")
                   for hh in range(HPC)] for b in range(B)]

            def outproj_chain(b, hh, cpi):
                csl = slice(cpi * TC_, (cpi + 1) * TC_)
                psy = psc_pool[0].tile([P, TC_], f32, tag="y")
                # stationary: oT columns {t : t%16 == j}, strided view
                ovw = oT[b][hh].rearrange("p (u j) -> p j u", j=KO)
                for j in range(KO):
                    nc.tensor.matmul(psy[:], ovw[:, j, :],
                                     wout_sb[:, j, csl],
                                     start=(j == 0), stop=(j == KO - 1))
                ysb = yp.tile([P, TC_], f32, tag="ysb")
                nc.scalar.copy(ysb[:], psy[:])
                nc.sync.dma_start(
                    y.ap()[b, hh * D:(hh + 1) * D, csl], ysb[:])

            psc_pool = [None]

            for b in range(B):
                qT = [qp.tile([P, T], bf16, tag=f"qT{hh}", name=f"qT{b}{hh}")
                      for hh in range(HPC)]
                kT = [qp.tile([P, T], bf16, tag=f"kT{hh}", name=f"kT{b}{hh}")
                      for hh in range(HPC)]
                vt = [qp.tile([P, NSC, D], bf16, tag=f"v{hh}", name=f"v{b}{hh}")
                      for hh in range(HPC)]

                # ---------------- QKV projection + RoPE ----------------
                with tc.tile_pool(name=f"xt{b}", bufs=3) as xp, \
                     tc.tile_pool(name=f"psA{b}", bufs=3, space="PSUM") as psa, \
                     tc.tile_pool(name=f"psV{b}", bufs=2, space="PSUM") as psv_p, \
                     tc.tile_pool(name=f"rope{b}", bufs=3) as rp:

                    def qkmm(xt, w_sb, hh, nm):
                        hsl = slice(hh * D, (hh + 1) * D)
                        ps = psa.tile([P, TQ], f32, tag="acc", name=nm)
                        for ko in range(KO):
                            nc.tensor.matmul(ps[:], w_sb[:, ko, hsl],
                                             xt[:, ko, :],
                                             start=(ko == 0),
                                             stop=(ko == KO - 1))
                        return ps

                    def rope(ps, dst, sl):
                        # tcos = ps * [cos;cos]; tsw pre-swaps halves:
                        # tsw[0:64]=q2*sin, tsw[64:128]=q1*sin so the add/sub
                        # reads align on base partitions.  All elementwise
                        # work on the VE (bf16 operands get 2x mode).
                        cs = cs_sb[:, sl]
                        sn = sn_sb[:, sl]
                        tcos = rp.tile([P, TQ], bf16, tag="tcos")
                        tsw = rp.tile([P, TQ], bf16, tag="tsw")
                        nc.vector.tensor_mul(tcos[:], ps[:], cs)
                        nc.vector.tensor_mul(tsw[0:HALF, :], ps[HALF:P, :], sn)
                        nc.vector.tensor_mul(tsw[HALF:P, :], ps[0:HALF, :], sn)
                        nc.vector.tensor_sub(dst[0:HALF, sl],
                                             tcos[0:HALF, :], tsw[0:HALF, :])
                        nc.vector.tensor_add(dst[HALF:P, sl],
                                             tcos[HALF:P, :], tsw[HALF:P, :])

                    def vchain(xt, ti):
                        for sub in range(TQ // P):
                            psv = psv_p.tile([P, HPC * D], f32, tag="acc")
                            for ko in range(KO):
                                nc.tensor.matmul(
                                    psv[:], xt[:, ko, sub * P:(sub + 1) * P],
                                    wv_sb[:, ko, :],
                                    start=(ko == 0), stop=(ko == KO - 1))
                            tci = ti * (TQ // P) + sub
                            for hh in range(HPC):
                                # Act engine is idle during QKV; it does the
                                # psum->sbuf v copies.
                                nc.scalar.copy(
                                    vt[hh][:, tci, :],
                                    psv[:, hh * D:(hh + 1) * D])

                    xts = {}
                    for ti in range(NT):
                        xts[ti] = xp.tile([P, KO, TQ], bf16, tag="xt",
                                          name=f"xt{b}_{ti}")

                    if b == 0:
                        # Startup is a DMA-bandwidth wall: ~7MB must land in
                        # the first ~30us.  Chunk the first two x tiles so
                        # chains pace behind arriving data, interleave wq/x
                        # chunks in ko-consumption order, and defer ti0's
                        # v-chains until after ti1's q/k so wv is needed
                        # later.
                        for g in range(4):
                            nc.sync.dma_start(
                                xts[0][:, 4 * g:4 * g + 4, :],
                                xTt.ap()[b, 0, :, 4 * g:4 * g + 4, :])
                        ps = qkmm(xts[0], wq_sb, 0, "acc0_q0")
                        nc.sync.dma_start(wk_sb[:], wk.ap())
                        nc.sync.dma_start(cs_sb[:], cs2.ap())
                        nc.sync.dma_start(sn_sb[:], sn1.ap())
                        rope(ps, qT[0], slice(0, TQ))
                        rope(qkmm(xts[0], wq_sb, 1, "acc0_q1"), qT[1],
                             slice(0, TQ))
                        for g in range(4):
                            nc.sync.dma_start(
                                xts[1][:, 4 * g:4 * g + 4, :],
                                xTt.ap()[b, 1, :, 4 * g:4 * g + 4, :])
                        rope(qkmm(xts[0], wk_sb, 0, "acc0_k0"), kT[0],
                             slice(0, TQ))
                        nc.sync.dma_start(wv_sb[:], wv.ap())
                        nc.sync.dma_start(mask_sb[:], maskM.ap())
                        rope(qkmm(xts[0], wk_sb, 1, "acc0_k1"), kT[1],
                             slice(0, TQ))
                        sl1 = slice(TQ, 2 * TQ)
                        rope(qkmm(xts[1], wq_sb, 0, "acc1_q0"), qT[0], sl1)
                        rope(qkmm(xts[1], wq_sb, 1, "acc1_q1"), qT[1], sl1)
                        rope(qkmm(xts[1], wk_sb, 0, "acc1_k0"), kT[0], sl1)
                        rope(qkmm(xts[1], wk_sb, 1, "acc1_k1"), kT[1], sl1)
                        vchain(xts[1], 1)
                        vchain(xts[0], 0)
                        rest = range(2, NT)
                    else:
                        rest = range(NT)

                    for ti in rest:
                        sl = slice(ti * TQ, (ti + 1) * TQ)
                        xt = xts[ti]
                        nc.sync.dma_start(xt[:], xTt.ap()[b, ti])
                        for hh in range(HPC):
                            rope(qkmm(xt, wq_sb, hh, f"a{ti}q{hh}"),
                                 qT[hh], sl)
                            rope(qkmm(xt, wk_sb, hh, f"a{ti}k{hh}"),
                                 kT[hh], sl)
                        vchain(xt, ti)

                # ------------- attention (S^T layout) + interleaved -----
                # ------------- out-projection of the previous head ------
                # s-chunks are processed in PAIRS sharing one 2-bank PSUM
                # tile and a single exp instruction, so the Act engine
                # (1024 cols + one fixed overhead) runs faster than the
                # PE's 6 matmuls per pair and never paces the pipeline.
                with tc.tile_pool(name=f"psBsc{b}", bufs=2, space="PSUM") as pssc, \
                     tc.tile_pool(name=f"psBo{b}", bufs=2, space="PSUM") as pso, \
                     tc.tile_pool(name=f"psBsum{b}", bufs=2, space="PSUM") as pssum, \
                     tc.tile_pool(name=f"pt{b}", bufs=4) as ptp:
                    nwo = 0   # wout prefetch cursor (b == 0 only)
                    # Deferred-issue queue: each t-tile's normalize is
                    # issued inside the NEXT tile's block loop (ps_o and
                    # ps_sum have bufs=2, so it must be issued before the
                    # slot cycles) -- the PE/Act pipeline never waits on it.
                    norm_q = []
                    # the score->PV pipeline is carried ACROSS (head,
                    # t-tile) boundaries: the next tile's score matmuls
                    # cover the previous tile's final flush, so no tile
                    # drains with an exp-wait bubble.  pend entries carry
                    # their own (ps_o, ps_sum, head, is-last-pair) context.
                    pend = []

                    def flush():
                        # same-accumulation-group matmuls must be adjacent:
                        # a LDWEIGHTS that follows an accumulating matmul
                        # whose group is being suspended stalls ~95ns on hw
                        # (after a STOPPED group it is free).  Flush up to
                        # TWO pairs at once, all o-matmuls in one burst
                        # then all sum-matmuls.
                        take, pend[:] = pend[:2], pend[2:]
                        for pt_, sws, ps_o_, ps_sum_, hh_, lastp in take:
                            for k, (s_, w_) in enumerate(sws):
                                nc.tensor.matmul(
                                    ps_o_[:, w_], vt[hh_][:, s_, :],
                                    pt_[:, k, w_], start=(s_ == 0),
                                    stop=(lastp and k == len(sws) - 1))
                        for pt_, sws, ps_o_, ps_sum_, hh_, lastp in take:
                            for k, (s_, w_) in enumerate(sws):
                                nc.tensor.matmul(
                                    ps_sum_[:, w_], ones_col[:],
                                    pt_[:, k, w_], start=(s_ == 0),
                                    stop=(lastp and k == len(sws) - 1))

                    for hh in range(HPC):
                        for ta in range(NTA):
                            spt = TA // P
                            tsl = slice(ta * TA, (ta + 1) * TA)
                            ps_o = pso.tile([P, TA], f32, tag="o")
                            ps_sum = pssum.tile([1, TA], f32, tag="sum")
                            nblk = (ta + 1) * spt

                            for pi in range(nblk // 2):
                                ps_sc = pssc.tile([P, 2, TA], f32, tag="sc")
                                pt = ptp.tile([P, 2, TA], bf16, tag="pt")
                                sws = []
                                for k in range(2):
                                    s = 2 * pi + k
                                    diag = s >= ta * spt
                                    t_lo = (s - ta * spt) * P if diag else 0
                                    w = slice(t_lo, TA)
                                    qsl = slice(ta * TA + t_lo,
                                                (ta + 1) * TA)
                                    nc.tensor.matmul(
                                        ps_sc[:, k, w],
                                        kT[hh][:, s * P:(s + 1) * P],
                                        qT[hh][:, qsl],
                                        start=True, stop=True)
                                    sws.append((s, w))
                                # one exp for both chunks; cols outside a
                                # diag chunk's window hold stale psum ->
                                # garbage pt that no matmul reads
                                nc.scalar.activation(pt[:, :, :],
                                                     ps_sc[:, :, :],
                                                     EXP, scale=SCALE)
                                for k, (s, w) in enumerate(sws):
                                    if s >= ta * spt:  # mask the triangle
                                        t_lo = (s - ta * spt) * P
                                        nc.vector.tensor_mul(
                                            pt[:, k, t_lo:t_lo + P],
                                            pt[:, k, t_lo:t_lo + P],
                                            mask_sb[:])
                                pend.append((pt, sws, ps_o, ps_sum, hh,
                                             pi == nblk // 2 - 1))
                                if len(pend) > 2:
                                    flush()
                                if pi == 0 and norm_q:
                                    norm_q.pop(0)()

                            # normalization, deferred: recip on VE,
                            # partition-broadcast on gpsimd, flat multiply
                            # on VE -- issued inside the next tile's block
                            # loop so the PE/Act pipeline never waits.
                            def normalize(ps_o=ps_o, ps_sum=ps_sum,
                                          hh=hh, tsl=tsl):
                                recf = sp.tile([1, TA], f32, tag="recf")
                                nc.vector.reciprocal_approx_fast(
                                    recf[:], ps_sum[:])
                                recb = sp.tile([1, TA], bf16, tag="recb")
                                nc.vector.tensor_copy(recb[:], recf[:])
                                bcb = sp.tile([P, TA], bf16, tag="bcb")
                                nc.gpsimd.partition_broadcast(bcb[:],
                                                              recb[:],
                                                              channels=P)
                                nc.vector.tensor_mul(oT[b][hh][:, tsl],
                                                     ps_o[:], bcb[:])
                            norm_q.append(normalize)

                            if b == 0 and hh == 0:
                                # prefetch all of wout during head 0's
                                # attention (the first out-proj chain needs
                                # every j block)
                                for _ in range(4):
                                    nc.sync.dma_start(
                                        wout_sb[:, nwo, :], woutT.ap()[nwo])
                                    nwo += 1

                    while pend:
                        flush()
                    for t_ in norm_q:
                        t_()

                # ---------------- output projection (pure PE) ----------
                with tc.tile_pool(name=f"psC{b}", bufs=2, space="PSUM") as psc:
                    psc_pool[0] = psc
                    for hh in range(HPC):
                        for cpi in range(NCP):
                            outproj_chain(b, hh, cpi)

    nc.compile()
    return nc


_NC = None


def _get_nc():
    global _NC
    if _NC is None:
        _NC = _build()
    return _NC


def _host_tables():
    pos = np.arange(T, dtype=np.float32)[:, None]
    div = np.exp(np.arange(0, 2 * HALF, 2, dtype=np.float32)
                 * np.float32(-math.log(ROPE_BASE) / (2 * HALF)))
    ang = pos * div[None, :]
    cosv = np.cos(ang).astype(np.float32)   # [T, HALF]
    sinv = np.sin(ang).astype(np.float32)
    cosT = np.ascontiguousarray(cosv.T)     # [HALF, T]
    sinT = np.ascontiguousarray(sinv.T)
    cs2 = np.ascontiguousarray(
        np.concatenate([cosT, cosT], axis=0)).astype(ml_dtypes.bfloat16)
    sn1 = np.ascontiguousarray(sinT).astype(ml_dtypes.bfloat16)
    # triangle mask M[s, w] = 1 iff s <= w
    ww = np.arange(P)[None, :]
    ss = np.arange(P)[:, None]
    maskM = (ss <= ww).astype(ml_dtypes.bfloat16)
    return cs2, sn1, maskM


def _make_in_maps(x, Wqkv, Wout):
    x = np.asarray(x, dtype=np.float32)
    Wqkv = np.asarray(Wqkv, dtype=np.float32)
    Wout = np.asarray(Wout, dtype=np.float32)
    assert x.shape == (B, T, C) and Wqkv.shape == (C, 3 * C) \
        and Wout.shape == (C, C)

    cs2, sn1, maskM = _host_tables()
    # xTt[b, ti, p, ko, u] = x[b, ti*TQ+u, ko*128+p]
    xTt = np.ascontiguousarray(
        x.reshape(B, NT, TQ, KO, P).transpose(0, 1, 4, 3, 2)
    ).astype(ml_dtypes.bfloat16)
    woutT = np.ascontiguousarray(
        Wout.reshape(KO, P, C)).astype(ml_dtypes.bfloat16)

    in_maps = []
    for core in range(NCORES):
        h0 = core * HPC
        cols = slice(h0 * D, (h0 + HPC) * D)
        ws = []
        for part in range(3):
            w = Wqkv[:, part * C:(part + 1) * C][:, cols]  # [C, HPC*D]
            ws.append(np.ascontiguousarray(
                w.reshape(KO, P, HPC * D).transpose(1, 0, 2)
            ).astype(ml_dtypes.bfloat16))
        in_maps.append({
            "xTt": xTt,
            "wq": ws[0], "wk": ws[1], "wv": ws[2],
            "woutT": woutT,
            "cs2": cs2, "sn1": sn1, "maskM": maskM,
        })
    return in_maps


def _run(x, Wqkv, Wout, trace=False):
    nc = _get_nc()
    in_maps = _make_in_maps(x, Wqkv, Wout)
    res = run_bass_kernel_spmd(nc, in_maps, core_ids=list(range(NCORES)),
                               trace=trace)
    out = np.empty((B, T, C), dtype=np.float32)
    for core in range(NCORES):
        out[:, core * HPC * D:(core + 1) * HPC * D, :] = \
            res.results[core]["y"]
    return out, res


def kernel(x, Wqkv, Wout):
    out, _ = _run(x, Wqkv, Wout)
    return out


# revision 45
# speedup vs baseline: 1.0331x; 1.0113x over previous
"""Trainium2 Bass kernel for nn_MultiHeadAttention_63015760167496.

Computation (see reference): qkv = x @ Wqkv; RoPE on q,k; causal softmax
attention per head; out = einsum('bhts,bshd->bhtd', probs, v);
out.reshape(B,T,C) @ Wout  -- the reshape is a *head-major* flatten of
[B,H,T,D] into [B,T,C], so final-output row r = h*128 + t//16 depends only
on head h.  Sharding: head-parallel over 8 cores (2 heads/core); every core
computes its two heads end-to-end and produces final-output rows
[256*i, 256*i+256).  Host concatenates -- no collectives.

All on-device data is bf16 (PSUM accumulation f32), which halves DMA/SBUF
vs f32r at the same 1 cycle/row PE rate.  Attention runs in S^T layout
([s,t]): softmax denominator via a ones-column matmul (partition reduction
on the PE), normalization broadcast via gpsimd.partition_broadcast and a
flat VE multiply, both issued *deferred* (inside the next t-tile's block
loop) so they never gate the PE.  The attention inner loop is
software-pipelined: score blocks are processed in pairs sharing one
two-bank PSUM tile and a single exp instruction, and the PV/sum matmuls of
the previous two pairs are flushed in same-accumulation-group bursts (a
LDWEIGHTS after an accumulating matmul whose group is suspended stalls
~95ns on hw).  O^T is stored flat [d, t]; the out-projection reads it
through a strided LDWEIGHTS view.  Wout is prefetched into SBUF during
attention b=0 so the out-projection phases are pure PE.
"""

import math
import sys

for _p in ("/opt/trn_rl_repo", "/root/.axon_site/_ro/trn_rl_repo"):
    if _p not in sys.path:
        sys.path.insert(0, _p)

import numpy as np
import ml_dtypes

import concourse.bass as bass
import concourse.mybir as mybir
import concourse.tile as tile
from concourse import bacc
from concourse.bass_utils import run_bass_kernel_spmd

B, T, C = 2, 2048, 2048
H = 16            # heads total
D = C // H        # 128 head dim
HALF = D // 2     # 64
P = 128
KO = C // P       # 16 contraction chunks
NCORES = 8
HPC = H // NCORES  # 2 heads per core
TQ = 512          # t-tile for qkv projection
NT = T // TQ      # 4
TA = 512          # t-tile for attention
NTA = T // TA     # 4
NSC = T // P      # 16 s-chunks
ROPE_BASE = 10000.0
SCALE = 1.0 / math.sqrt(D)
TC_ = 512         # col-tile for out projection
NCP = C // TC_    # 4
LOOKAHEAD = 2     # attention software-pipeline depth

f32 = mybir.dt.float32
bf16 = mybir.dt.bfloat16
EXP = mybir.ActivationFunctionType.Exp


def _build():
    nc = bacc.Bacc("TRN2", target_bir_lowering=False, debug=False,
                   num_devices=NCORES)

    # host-pre-tiled x^T: xTt[b, ti, p, ko, u] = x[b, ti*TQ+u, ko*128+p]
    xTt = nc.dram_tensor("xTt", [B, NT, P, KO, TQ], bf16, kind="ExternalInput")
    # host-pre-chunked weights: w[p, ko, m] = W[ko*128+p, m]
    wq = nc.dram_tensor("wq", [P, KO, HPC * D], bf16, kind="ExternalInput")
    wk = nc.dram_tensor("wk", [P, KO, HPC * D], bf16, kind="ExternalInput")
    wv = nc.dram_tensor("wv", [P, KO, HPC * D], bf16, kind="ExternalInput")
    # woutT[j, p, c] = Wout[j*128+p, c]
    woutT = nc.dram_tensor("woutT", [KO, P, C], bf16, kind="ExternalInput")
    cs2 = nc.dram_tensor("cs2", [P, T], bf16, kind="ExternalInput")  # [cos;cos]
    sn1 = nc.dram_tensor("sn1", [HALF, T], bf16, kind="ExternalInput")  # sin
    maskM = nc.dram_tensor("maskM", [P, P], bf16, kind="ExternalInput")
    y = nc.dram_tensor("y", [B, HPC * D, C], f32, kind="ExternalOutput")

    with tile.TileContext(nc) as tc:
        with tc.tile_pool(name="const", bufs=1) as cp_, \
             tc.tile_pool(name="wo", bufs=1) as wop, \
             tc.tile_pool(name="qkv", bufs=1) as qp, \
             tc.tile_pool(name="ot", bufs=1) as op_, \
             tc.tile_pool(name="ys", bufs=4) as yp, \
             tc.tile_pool(name="small", bufs=2) as sp:

            wq_sb = cp_.tile([P, KO, HPC * D], bf16, tag="wq")
            wk_sb = cp_.tile([P, KO, HPC * D], bf16, tag="wk")
            wv_sb = cp_.tile([P, KO, HPC * D], bf16, tag="wv")
            cs_sb = cp_.tile([P, T], bf16, tag="cs")
            sn_sb = cp_.tile([HALF, T], bf16, tag="sn")
            mask_sb = cp_.tile([P, P], bf16, tag="mask")
            wout_sb = wop.tile([P, KO, C], bf16, tag="wout")

            # startup DMAs: wq first (chunked) so the first chain starts
            # ASAP; the first chunk goes through the gpsimd DGE (its
            # framework preamble ends ~2.5us before the sync engine's).
            nc.gpsimd.dma_start(wq_sb[:, 0:8, :], wq.ap()[:, 0:8, :])
            nc.sync.dma_start(wq_sb[:, 8:16, :], wq.ap()[:, 8:16, :])

            ones_f32 = cp_.tile([P, 1], f32, tag="ones_f32")
            nc.vector.memset(ones_f32[:], 1.0)
            ones_col = cp_.tile([P, 1], bf16, tag="ones_col")
            nc.vector.tensor_copy(ones_col[:], ones_f32[:])
            # act-table warmup: force the Exp table load at t=0 instead of
            # in the middle of the first attention block.
            warm_in = cp_.tile([1, 8], f32, tag="warm_in")
            nc.vector.memset(warm_in[:], 0.0)
            warm_out = cp_.tile([1, 8], f32, tag="warm_out")
            nc.scalar.activation(warm_out[:], warm_in[:], EXP, scale=1.0)

            # persistent attention outputs O^T per (b, local head): [d, t]
            oT = [[op_.tile([P, T], bf16, tag=f"oT{b}{hh}", name=f"oT{b}{hh}")
                   for hh in range(HPC)] for b in range(B)]

            def outproj_chain(b, hh, cpi):
                csl = slice(cpi * TC_, (cpi + 1) * TC_)
                psy = psc_pool[0].tile([P, TC_], f32, tag="y")
                # stationary: oT columns {t : t%16 == j}, strided view
                ovw = oT[b][hh].rearrange("p (u j) -> p j u", j=KO)
                for j in range(KO):
                    nc.tensor.matmul(psy[:], ovw[:, j, :],
                                     wout_sb[:, j, csl],
                                     start=(j == 0), stop=(j == KO - 1))
                ysb = yp.tile([P, TC_], f32, tag="ysb")
                nc.scalar.copy(ysb[:], psy[:])
                nc.sync.dma_start(
                    y.ap()[b, hh * D:(hh + 1) * D, csl], ysb[:])

            psc_pool = [None]

            for b in range(B):
                qT = [qp.tile([P, T], bf16, tag=f"qT{hh}", name=f"qT{b}{hh}")
                      for hh in range(HPC)]
                kT = [qp.tile([P, T], bf16, tag=f"kT{hh}", name=f"kT{b}{hh}")
                      for hh in range(HPC)]
                vt = [qp.tile([P, NSC, D], bf16, tag=f"v{hh}", name=f"v{b}{hh}")
                      for hh in range(HPC)]

                # ---------------- QKV projection + RoPE ----------------
                with tc.tile_pool(name=f"xt{b}", bufs=3) as xp, \
                     tc.tile_pool(name=f"psA{b}", bufs=3, space="PSUM") as psa, \
                     tc.tile_pool(name=f"psV{b}", bufs=2, space="PSUM") as psv_p, \
                     tc.tile_pool(name=f"rope{b}", bufs=3) as rp:

                    def qkmm(xt, w_sb, hh, nm):
                        hsl = slice(hh * D, (hh + 1) * D)
                        ps = psa.tile([P, TQ], f32, tag="acc", name=nm)
                        for ko in range(KO):
                            nc.tensor.matmul(ps[:], w_sb[:, ko, hsl],
                                             xt[:, ko, :],
                                             start=(ko == 0),
                                             stop=(ko == KO - 1))
                        return ps

                    def rope(ps, dst, sl):
                        # tcos = ps * [cos;cos]; tsw pre-swaps halves:
                        # tsw[0:64]=q2*sin, tsw[64:128]=q1*sin so the add/sub
                        # reads align on base partitions.  All elementwise
                        # work on the VE (bf16 operands get 2x mode).
                        cs = cs_sb[:, sl]
                        sn = sn_sb[:, sl]
                        tcos = rp.tile([P, TQ], bf16, tag="tcos")
                        tsw = rp.tile([P, TQ], bf16, tag="tsw")
                        nc.vector.tensor_mul(tcos[:], ps[:], cs)
                        nc.vector.tensor_mul(tsw[0:HALF, :], ps[HALF:P, :], sn)
                        nc.vector.tensor_mul(tsw[HALF:P, :], ps[0:HALF, :], sn)
                        nc.vector.tensor_sub(dst[0:HALF, sl],
                                             tcos[0:HALF, :], tsw[0:HALF, :])
                        nc.vector.tensor_add(dst[HALF:P, sl],
                                             tcos[HALF:P, :], tsw[HALF:P, :])

                    def vchain(xt, ti):
                        for sub in range(TQ // P):
                            psv = psv_p.tile([P, HPC * D], f32, tag="acc")
                            for ko in range(KO):
                                nc.tensor.matmul(
                                    psv[:], xt[:, ko, sub * P:(sub + 1) * P],
                                    wv_sb[:, ko, :],
                                    start=(ko == 0), stop=(ko == KO - 1))
                            tci = ti * (TQ // P) + sub
                            for hh in range(HPC):
                                # Act engine is idle during QKV; it does the
                                # psum->sbuf v copies.
                                nc.scalar.copy(
                                    vt[hh][:, tci, :],
                                    psv[:, hh * D:(hh + 1) * D])

                    xts = {}
                    for ti in range(NT):
                        xts[ti] = xp.tile([P, KO, TQ], bf16, tag="xt",
                                          name=f"xt{b}_{ti}")

                    if b == 0:
                        # Startup is a DMA-bandwidth wall: ~7MB must land in
                        # the first ~30us.  Chunk the first two x tiles so
                        # chains pace behind arriving data, interleave wq/x
                        # chunks in ko-consumption order, and defer ti0's
                        # v-chains until after ti1's q/k so wv is needed
                        # later.
                        for g in range(4):
                            nc.sync.dma_start(
                                xts[0][:, 4 * g:4 * g + 4, :],
                                xTt.ap()[b, 0, :, 4 * g:4 * g + 4, :])
                        ps = qkmm(xts[0], wq_sb, 0, "acc0_q0")
                        nc.sync.dma_start(wk_sb[:], wk.ap())
                        nc.sync.dma_start(cs_sb[:], cs2.ap())
                        nc.sync.dma_start(sn_sb[:], sn1.ap())
                        rope(ps, qT[0], slice(0, TQ))
                        rope(qkmm(xts[0], wq_sb, 1, "acc0_q1"), qT[1],
                             slice(0, TQ))
                        for g in range(4):
                            nc.sync.dma_start(
                                xts[1][:, 4 * g:4 * g + 4, :],
                                xTt.ap()[b, 1, :, 4 * g:4 * g + 4, :])
                        rope(qkmm(xts[0], wk_sb, 0, "acc0_k0"), kT[0],
                             slice(0, TQ))
                        nc.sync.dma_start(wv_sb[:], wv.ap())
                        nc.sync.dma_start(mask_sb[:], maskM.ap())
                        rope(qkmm(xts[0], wk_sb, 1, "acc0_k1"), kT[1],
                             slice(0, TQ))
                        sl1 = slice(TQ, 2 * TQ)
                        rope(qkmm(xts[1], wq_sb, 0, "acc1_q0"), qT[0], sl1)
                        rope(qkmm(xts[1], wq_sb, 1, "acc1_q1"), qT[1], sl1)
                        rope(qkmm(xts[1], wk_sb, 0, "acc1_k0"), kT[0], sl1)
                        rope(qkmm(xts[1], wk_sb, 1, "acc1_k1"), kT[1], sl1)
                        vchain(xts[1], 1)
                        vchain(xts[0], 0)
                        rest = range(2, NT)
                    else:
                        rest = range(NT)

                    for ti in rest:
                        sl = slice(ti * TQ, (ti + 1) * TQ)
                        xt = xts[ti]
                        nc.sync.dma_start(xt[:], xTt.ap()[b, ti])
                        for hh in range(HPC):
                            rope(qkmm(xt, wq_sb, hh, f"a{ti}q{hh}"),
                                 qT[hh], sl)
                            rope(qkmm(xt, wk_sb, hh, f"a{ti}k{hh}"),
                                 kT[hh], sl)
                        vchain(xt, ti)

                # ------------- attention (S^T layout) + interleaved -----
                # ------------- out-projection of the previous head ------
                # s-chunks are processed in PAIRS sharing one 2-bank PSUM
                # tile and a single exp instruction, so the Act engine
                # (1024 cols + one fixed overhead) runs faster than the
                # PE's 6 matmuls per pair and never paces the pipeline.
                with tc.tile_pool(name=f"psBsc{b}", bufs=2, space="PSUM") as pssc, \
                     tc.tile_pool(name=f"psBo{b}", bufs=2, space="PSUM") as pso, \
                     tc.tile_pool(name=f"psBsum{b}", bufs=2, space="PSUM") as pssum, \
                     tc.tile_pool(name=f"pt{b}", bufs=4) as ptp, \
                     tc.tile_pool(name=f"pts{b}", bufs=3) as ptsp:
                    nwo = 0   # wout prefetch cursor (b == 0 only)
                    # Deferred-issue queue: each t-tile's normalize is
                    # issued inside the NEXT tile's block loop (ps_o and
                    # ps_sum have bufs=2, so it must be issued before the
                    # slot cycles) -- the PE/Act pipeline never waits on it.
                    norm_q = []
                    # the score->PV pipeline is carried ACROSS (head,
                    # t-tile) boundaries: the next tile's score matmuls
                    # cover the previous tile's final flush, so no tile
                    # drains with an exp-wait bubble.  pend entries carry
                    # their own (ps_o, ps_sum, head, is-last-pair) context.
                    pend = []

                    def flush():
                        # same-accumulation-group matmuls must be adjacent:
                        # a LDWEIGHTS that follows an accumulating matmul
                        # whose group is being suspended stalls ~95ns on hw
                        # (after a STOPPED group it is free).  Flush up to
                        # TWO pairs at once, all o-matmuls in one burst
                        # then all sum-matmuls.  Non-diag pairs carry a
                        # VE-pre-added pair-sum tile, so their softmax
                        # denominator needs ONE ones-matmul, not two.
                        take, pend[:] = pend[:2], pend[2:]
                        for pt_, sws, ps_o_, ps_sum_, hh_, lastp, _ in take:
                            for k, (s_, w_) in enumerate(sws):
                                nc.tensor.matmul(
                                    ps_o_[:, w_], vt[hh_][:, s_, :],
                                    pt_[:, k, w_], start=(s_ == 0),
                                    stop=(lastp and k == len(sws) - 1))
                        for pt_, sws, ps_o_, ps_sum_, hh_, lastp, pts in take:
                            if pts is not None:
                                nc.tensor.matmul(
                                    ps_sum_[:, :], ones_col[:], pts[:],
                                    start=(sws[0][0] == 0), stop=False)
                                continue
                            for k, (s_, w_) in enumerate(sws):
                                nc.tensor.matmul(
                                    ps_sum_[:, w_], ones_col[:],
                                    pt_[:, k, w_], start=(s_ == 0),
                                    stop=(lastp and k == len(sws) - 1))

                    for hh in range(HPC):
                        for ta in range(NTA):
                            spt = TA // P
                            tsl = slice(ta * TA, (ta + 1) * TA)
                            ps_o = pso.tile([P, TA], f32, tag="o")
                            ps_sum = pssum.tile([1, TA], f32, tag="sum")
                            nblk = (ta + 1) * spt

                            for pi in range(nblk // 2):
                                ps_sc = pssc.tile([P, 2, TA], f32, tag="sc")
                                pt = ptp.tile([P, 2, TA], bf16, tag="pt")
                                sws = []
                                for k in range(2):
                                    s = 2 * pi + k
                                    diag = s >= ta * spt
                                    t_lo = (s - ta * spt) * P if diag else 0
                                    w = slice(t_lo, TA)
                                    qsl = slice(ta * TA + t_lo,
                                                (ta + 1) * TA)
                                    nc.tensor.matmul(
                                        ps_sc[:, k, w],
                                        kT[hh][:, s * P:(s + 1) * P],
                                        qT[hh][:, qsl],
                                        start=True, stop=True)
                                    sws.append((s, w))
                                # one exp for both chunks; cols outside a
                                # diag chunk's window hold stale psum ->
                                # garbage pt that no matmul reads
                                nc.scalar.activation(pt[:, :, :],
                                                     ps_sc[:, :, :],
                                                     EXP, scale=SCALE)
                                for k, (s, w) in enumerate(sws):
                                    if s >= ta * spt:  # mask the triangle
                                        t_lo = (s - ta * spt) * P
                                        nc.vector.tensor_mul(
                                            pt[:, k, t_lo:t_lo + P],
                                            pt[:, k, t_lo:t_lo + P],
                                            mask_sb[:])
                                pts = None
                                if pi < 2 * ta:  # non-diag pair
                                    pts = ptsp.tile([P, TA], bf16,
                                                    tag="ptsum")
                                    nc.vector.tensor_add(pts[:],
                                                         pt[:, 0, :],
                                                         pt[:, 1, :])
                                pend.append((pt, sws, ps_o, ps_sum, hh,
                                             pi == nblk // 2 - 1, pts))
                                if len(pend) > 2:
                                    flush()
                                if pi == 0 and norm_q:
                                    norm_q.pop(0)()

                            # normalization, deferred: recip on VE,
                            # partition-broadcast on gpsimd, flat multiply
                            # on VE -- issued inside the next tile's block
                            # loop so the PE/Act pipeline never waits.
                            def normalize(ps_o=ps_o, ps_sum=ps_sum,
                                          hh=hh, tsl=tsl):
                                recf = sp.tile([1, TA], f32, tag="recf")
                                nc.vector.reciprocal_approx_fast(
                                    recf[:], ps_sum[:])
                                recb = sp.tile([1, TA], bf16, tag="recb")
                                nc.vector.tensor_copy(recb[:], recf[:])
                                bcb = sp.tile([P, TA], bf16, tag="bcb")
                                nc.gpsimd.partition_broadcast(bcb[:],
                                                              recb[:],
                                                              channels=P)
                                nc.vector.tensor_mul(oT[b][hh][:, tsl],
                                                     ps_o[:], bcb[:])
                            norm_q.append(normalize)

                            if b == 0 and hh == 0:
                                # prefetch all of wout during head 0's
                                # attention (the first out-proj chain needs
                                # every j block)
                                for _ in range(4):
                                    nc.sync.dma_start(
                                        wout_sb[:, nwo, :], woutT.ap()[nwo])
                                    nwo += 1

                    while pend:
                        flush()
                    for t_ in norm_q:
                        t_()

                # ---------------- output projection (pure PE) ----------
                with tc.tile_pool(name=f"psC{b}", bufs=2, space="PSUM") as psc:
                    psc_pool[0] = psc
                    for hh in range(HPC):
                        for cpi in range(NCP):
                            outproj_chain(b, hh, cpi)

    nc.compile()
    return nc


_NC = None


def _get_nc():
    global _NC
    if _NC is None:
        _NC = _build()
    return _NC


def _host_tables():
    pos = np.arange(T, dtype=np.float32)[:, None]
    div = np.exp(np.arange(0, 2 * HALF, 2, dtype=np.float32)
                 * np.float32(-math.log(ROPE_BASE) / (2 * HALF)))
    ang = pos * div[None, :]
    cosv = np.cos(ang).astype(np.float32)   # [T, HALF]
    sinv = np.sin(ang).astype(np.float32)
    cosT = np.ascontiguousarray(cosv.T)     # [HALF, T]
    sinT = np.ascontiguousarray(sinv.T)
    cs2 = np.ascontiguousarray(
        np.concatenate([cosT, cosT], axis=0)).astype(ml_dtypes.bfloat16)
    sn1 = np.ascontiguousarray(sinT).astype(ml_dtypes.bfloat16)
    # triangle mask M[s, w] = 1 iff s <= w
    ww = np.arange(P)[None, :]
    ss = np.arange(P)[:, None]
    maskM = (ss <= ww).astype(ml_dtypes.bfloat16)
    return cs2, sn1, maskM


def _make_in_maps(x, Wqkv, Wout):
    x = np.asarray(x, dtype=np.float32)
    Wqkv = np.asarray(Wqkv, dtype=np.float32)
    Wout = np.asarray(Wout, dtype=np.float32)
    assert x.shape == (B, T, C) and Wqkv.shape == (C, 3 * C) \
        and Wout.shape == (C, C)

    cs2, sn1, maskM = _host_tables()
    # xTt[b, ti, p, ko, u] = x[b, ti*TQ+u, ko*128+p]
    xTt = np.ascontiguousarray(
        x.reshape(B, NT, TQ, KO, P).transpose(0, 1, 4, 3, 2)
    ).astype(ml_dtypes.bfloat16)
    woutT = np.ascontiguousarray(
        Wout.reshape(KO, P, C)).astype(ml_dtypes.bfloat16)

    in_maps = []
    for core in range(NCORES):
        h0 = core * HPC
        cols = slice(h0 * D, (h0 + HPC) * D)
        ws = []
        for part in range(3):
            w = Wqkv[:, part * C:(part + 1) * C][:, cols]  # [C, HPC*D]
            ws.append(np.ascontiguousarray(
                w.reshape(KO, P, HPC * D).transpose(1, 0, 2)
            ).astype(ml_dtypes.bfloat16))
        in_maps.append({
            "xTt": xTt,
            "wq": ws[0], "wk": ws[1], "wv": ws[2],
            "woutT": woutT,
            "cs2": cs2, "sn1": sn1, "maskM": maskM,
        })
    return in_maps


def _run(x, Wqkv, Wout, trace=False):
    nc = _get_nc()
    in_maps = _make_in_maps(x, Wqkv, Wout)
    res = run_bass_kernel_spmd(nc, in_maps, core_ids=list(range(NCORES)),
                               trace=trace)
    out = np.empty((B, T, C), dtype=np.float32)
    for core in range(NCORES):
        out[:, core * HPC * D:(core + 1) * HPC * D, :] = \
            res.results[core]["y"]
    return out, res


def kernel(x, Wqkv, Wout):
    out, _ = _run(x, Wqkv, Wout)
    return out


# revision 47
# speedup vs baseline: 1.0384x; 1.0051x over previous
"""Trainium2 Bass kernel for nn_MultiHeadAttention_63015760167496.

Computation (see reference): qkv = x @ Wqkv; RoPE on q,k; causal softmax
attention per head; out = einsum('bhts,bshd->bhtd', probs, v);
out.reshape(B,T,C) @ Wout  -- the reshape is a *head-major* flatten of
[B,H,T,D] into [B,T,C], so final-output row r = h*128 + t//16 depends only
on head h.  Sharding: head-parallel over 8 cores (2 heads/core); every core
computes its two heads end-to-end and produces final-output rows
[256*i, 256*i+256).  Host concatenates -- no collectives.

All on-device data is bf16 (PSUM accumulation f32), which halves DMA/SBUF
vs f32r at the same 1 cycle/row PE rate.  Attention runs in S^T layout
([s,t]): softmax denominator via a ones-column matmul (partition reduction
on the PE), normalization broadcast via gpsimd.partition_broadcast and a
flat VE multiply, both issued *deferred* (inside the next t-tile's block
loop) so they never gate the PE.  The attention inner loop is
software-pipelined: score blocks are processed in pairs sharing one
two-bank PSUM tile and a single exp instruction, and the PV/sum matmuls of
the previous two pairs are flushed in same-accumulation-group bursts (a
LDWEIGHTS after an accumulating matmul whose group is suspended stalls
~95ns on hw).  O^T is stored flat [d, t]; the out-projection reads it
through a strided LDWEIGHTS view.  Wout is prefetched into SBUF during
attention b=0 so the out-projection phases are pure PE.
"""

import math
import sys

for _p in ("/opt/trn_rl_repo", "/root/.axon_site/_ro/trn_rl_repo"):
    if _p not in sys.path:
        sys.path.insert(0, _p)

import numpy as np
import ml_dtypes

import concourse.bass as bass
import concourse.mybir as mybir
import concourse.tile as tile
from concourse import bacc
from concourse.bass_utils import run_bass_kernel_spmd

B, T, C = 2, 2048, 2048
H = 16            # heads total
D = C // H        # 128 head dim
HALF = D // 2     # 64
P = 128
KO = C // P       # 16 contraction chunks
NCORES = 8
HPC = H // NCORES  # 2 heads per core
TQ = 512          # t-tile for qkv projection
NT = T // TQ      # 4
TA = 512          # t-tile for attention
NTA = T // TA     # 4
NSC = T // P      # 16 s-chunks
ROPE_BASE = 10000.0
SCALE = 1.0 / math.sqrt(D)
TC_ = 512         # col-tile for out projection
NCP = C // TC_    # 4
LOOKAHEAD = 2     # attention software-pipeline depth

f32 = mybir.dt.float32
bf16 = mybir.dt.bfloat16
EXP = mybir.ActivationFunctionType.Exp


def _build():
    nc = bacc.Bacc("TRN2", target_bir_lowering=False, debug=False,
                   num_devices=NCORES)

    # host-pre-tiled x^T: xTt[b, ti, p, ko, u] = x[b, ti*TQ+u, ko*128+p]
    xTt = nc.dram_tensor("xTt", [B, NT, P, KO, TQ], bf16, kind="ExternalInput")
    # host-pre-chunked weights: w[p, ko, m] = W[ko*128+p, m]
    wq = nc.dram_tensor("wq", [P, KO, HPC * D], bf16, kind="ExternalInput")
    wk = nc.dram_tensor("wk", [P, KO, HPC * D], bf16, kind="ExternalInput")
    wv = nc.dram_tensor("wv", [P, KO, HPC * D], bf16, kind="ExternalInput")
    # woutT[j, p, c] = Wout[j*128+p, c]
    woutT = nc.dram_tensor("woutT", [KO, P, C], bf16, kind="ExternalInput")
    cs2 = nc.dram_tensor("cs2", [P, T], bf16, kind="ExternalInput")  # [cos;cos]
    sn1 = nc.dram_tensor("sn1", [HALF, T], bf16, kind="ExternalInput")  # sin
    maskM = nc.dram_tensor("maskM", [P, P], bf16, kind="ExternalInput")
    y = nc.dram_tensor("y", [B, HPC * D, C], f32, kind="ExternalOutput")

    with tile.TileContext(nc) as tc:
        with tc.tile_pool(name="const", bufs=1) as cp_, \
             tc.tile_pool(name="wo", bufs=1) as wop, \
             tc.tile_pool(name="qkv", bufs=1) as qp, \
             tc.tile_pool(name="ot", bufs=1) as op_, \
             tc.tile_pool(name="ys", bufs=4) as yp, \
             tc.tile_pool(name="small", bufs=2) as sp:

            wq_sb = cp_.tile([P, KO, HPC * D], bf16, tag="wq")
            wk_sb = cp_.tile([P, KO, HPC * D], bf16, tag="wk")
            wv_sb = cp_.tile([P, KO, HPC * D], bf16, tag="wv")
            cs_sb = cp_.tile([P, T], bf16, tag="cs")
            sn_sb = cp_.tile([HALF, T], bf16, tag="sn")
            mask_sb = cp_.tile([P, P], bf16, tag="mask")
            wout_sb = wop.tile([P, KO, C], bf16, tag="wout")

            # startup DMAs: wq first (chunked) so the first chain starts
            # ASAP; the first chunk goes through the gpsimd DGE (its
            # framework preamble ends ~2.5us before the sync engine's).
            nc.gpsimd.dma_start(wq_sb[:, 0:8, :], wq.ap()[:, 0:8, :])
            nc.sync.dma_start(wq_sb[:, 8:16, :], wq.ap()[:, 8:16, :])

            ones_f32 = cp_.tile([P, 1], f32, tag="ones_f32")
            nc.vector.memset(ones_f32[:], 1.0)
            ones_col = cp_.tile([P, 1], bf16, tag="ones_col")
            nc.vector.tensor_copy(ones_col[:], ones_f32[:])
            # act-table warmup: force the Exp table load at t=0 instead of
            # in the middle of the first attention block.
            warm_in = cp_.tile([1, 8], f32, tag="warm_in")
            nc.vector.memset(warm_in[:], 0.0)
            warm_out = cp_.tile([1, 8], f32, tag="warm_out")
            nc.scalar.activation(warm_out[:], warm_in[:], EXP, scale=1.0)

            # persistent attention outputs O^T per (b, local head): [d, t]
            oT = [[op_.tile([P, T], bf16, tag=f"oT{b}{hh}", name=f"oT{b}{hh}")
                   for hh in range(HPC)] for b in range(B)]

            def outproj_chain(b, hh, cpi):
                csl = slice(cpi * TC_, (cpi + 1) * TC_)
                psy = psc_pool[0].tile([P, TC_], f32, tag="y")
                # stationary: oT columns {t : t%16 == j}, strided view
                ovw = oT[b][hh].rearrange("p (u j) -> p j u", j=KO)
                for j in range(KO):
                    nc.tensor.matmul(psy[:], ovw[:, j, :],
                                     wout_sb[:, j, csl],
                                     start=(j == 0), stop=(j == KO - 1))
                ysb = yp.tile([P, TC_], f32, tag="ysb")
                nc.scalar.copy(ysb[:], psy[:])
                nc.sync.dma_start(
                    y.ap()[b, hh * D:(hh + 1) * D, csl], ysb[:])

            psc_pool = [None]

            for b in range(B):
                qT = [qp.tile([P, T], bf16, tag=f"qT{hh}", name=f"qT{b}{hh}")
                      for hh in range(HPC)]
                kT = [qp.tile([P, T], bf16, tag=f"kT{hh}", name=f"kT{b}{hh}")
                      for hh in range(HPC)]
                vt = [qp.tile([P, NSC, D], bf16, tag=f"v{hh}", name=f"v{b}{hh}")
                      for hh in range(HPC)]

                # ---------------- QKV projection + RoPE ----------------
                with tc.tile_pool(name=f"xt{b}", bufs=3) as xp, \
                     tc.tile_pool(name=f"psA{b}", bufs=3, space="PSUM") as psa, \
                     tc.tile_pool(name=f"psV{b}", bufs=2, space="PSUM") as psv_p, \
                     tc.tile_pool(name=f"rope{b}", bufs=3) as rp:

                    def qkmm(xt, w_sb, hh, nm):
                        hsl = slice(hh * D, (hh + 1) * D)
                        ps = psa.tile([P, TQ], f32, tag="acc", name=nm)
                        for ko in range(KO):
                            nc.tensor.matmul(ps[:], w_sb[:, ko, hsl],
                                             xt[:, ko, :],
                                             start=(ko == 0),
                                             stop=(ko == KO - 1))
                        return ps

                    def rope(ps, dst, sl):
                        # tcos = ps * [cos;cos]; tsw pre-swaps halves:
                        # tsw[0:64]=q2*sin, tsw[64:128]=q1*sin so the add/sub
                        # reads align on base partitions.  All elementwise
                        # work on the VE (bf16 operands get 2x mode).
                        cs = cs_sb[:, sl]
                        sn = sn_sb[:, sl]
                        tcos = rp.tile([P, TQ], bf16, tag="tcos")
                        tsw = rp.tile([P, TQ], bf16, tag="tsw")
                        nc.vector.tensor_mul(tcos[:], ps[:], cs)
                        nc.vector.tensor_mul(tsw[0:HALF, :], ps[HALF:P, :], sn)
                        nc.vector.tensor_mul(tsw[HALF:P, :], ps[0:HALF, :], sn)
                        nc.vector.tensor_sub(dst[0:HALF, sl],
                                             tcos[0:HALF, :], tsw[0:HALF, :])
                        nc.vector.tensor_add(dst[HALF:P, sl],
                                             tcos[HALF:P, :], tsw[HALF:P, :])

                    def vchain(xt, ti):
                        for sub in range(TQ // P):
                            psv = psv_p.tile([P, HPC * D], f32, tag="acc")
                            for ko in range(KO):
                                nc.tensor.matmul(
                                    psv[:], xt[:, ko, sub * P:(sub + 1) * P],
                                    wv_sb[:, ko, :],
                                    start=(ko == 0), stop=(ko == KO - 1))
                            tci = ti * (TQ // P) + sub
                            for hh in range(HPC):
                                # Act engine is idle during QKV; it does the
                                # psum->sbuf v copies.
                                nc.scalar.copy(
                                    vt[hh][:, tci, :],
                                    psv[:, hh * D:(hh + 1) * D])

                    xts = {}
                    for ti in range(NT):
                        xts[ti] = xp.tile([P, KO, TQ], bf16, tag="xt",
                                          name=f"xt{b}_{ti}")

                    if b == 0:
                        # Startup is a DMA-bandwidth wall: ~7MB must land in
                        # the first ~30us.  Chunk the first two x tiles so
                        # chains pace behind arriving data, interleave wq/x
                        # chunks in ko-consumption order, and defer ti0's
                        # v-chains until after ti1's q/k so wv is needed
                        # later.
                        for g in range(4):
                            nc.sync.dma_start(
                                xts[0][:, 4 * g:4 * g + 4, :],
                                xTt.ap()[b, 0, :, 4 * g:4 * g + 4, :])
                        ps = qkmm(xts[0], wq_sb, 0, "acc0_q0")
                        nc.sync.dma_start(wk_sb[:], wk.ap())
                        nc.sync.dma_start(cs_sb[:], cs2.ap())
                        nc.sync.dma_start(sn_sb[:], sn1.ap())
                        rope(ps, qT[0], slice(0, TQ))
                        rope(qkmm(xts[0], wq_sb, 1, "acc0_q1"), qT[1],
                             slice(0, TQ))
                        for g in range(4):
                            nc.sync.dma_start(
                                xts[1][:, 4 * g:4 * g + 4, :],
                                xTt.ap()[b, 1, :, 4 * g:4 * g + 4, :])
                        rope(qkmm(xts[0], wk_sb, 0, "acc0_k0"), kT[0],
                             slice(0, TQ))
                        nc.sync.dma_start(wv_sb[:], wv.ap())
                        nc.sync.dma_start(mask_sb[:], maskM.ap())
                        rope(qkmm(xts[0], wk_sb, 1, "acc0_k1"), kT[1],
                             slice(0, TQ))
                        sl1 = slice(TQ, 2 * TQ)
                        rope(qkmm(xts[1], wq_sb, 0, "acc1_q0"), qT[0], sl1)
                        rope(qkmm(xts[1], wq_sb, 1, "acc1_q1"), qT[1], sl1)
                        rope(qkmm(xts[1], wk_sb, 0, "acc1_k0"), kT[0], sl1)
                        rope(qkmm(xts[1], wk_sb, 1, "acc1_k1"), kT[1], sl1)
                        vchain(xts[1], 1)
                        vchain(xts[0], 0)
                        rest = range(2, NT)
                    else:
                        rest = range(NT)

                    for ti in rest:
                        sl = slice(ti * TQ, (ti + 1) * TQ)
                        xt = xts[ti]
                        nc.sync.dma_start(xt[:], xTt.ap()[b, ti])
                        for hh in range(HPC):
                            rope(qkmm(xt, wq_sb, hh, f"a{ti}q{hh}"),
                                 qT[hh], sl)
                            rope(qkmm(xt, wk_sb, hh, f"a{ti}k{hh}"),
                                 kT[hh], sl)
                        vchain(xt, ti)

                # ------------- attention (S^T layout) + interleaved -----
                # ------------- out-projection of the previous head ------
                # s-chunks are processed in PAIRS sharing one 2-bank PSUM
                # tile and a single exp instruction, so the Act engine
                # (1024 cols + one fixed overhead) runs faster than the
                # PE's 6 matmuls per pair and never paces the pipeline.
                with tc.tile_pool(name=f"psBsc{b}", bufs=2, space="PSUM") as pssc, \
                     tc.tile_pool(name=f"psBo{b}", bufs=2, space="PSUM") as pso, \
                     tc.tile_pool(name=f"psBsum{b}", bufs=2, space="PSUM") as pssum, \
                     tc.tile_pool(name=f"pt{b}", bufs=4) as ptp, \
                     tc.tile_pool(name=f"pts{b}", bufs=4) as ptsp:
                    nwo = 0   # wout prefetch cursor (b == 0 only)
                    # Deferred-issue queue: each t-tile's normalize is
                    # issued inside the NEXT tile's block loop (ps_o and
                    # ps_sum have bufs=2, so it must be issued before the
                    # slot cycles) -- the PE/Act pipeline never waits on it.
                    norm_q = []
                    # the score->PV pipeline is carried ACROSS (head,
                    # t-tile) boundaries: the next tile's score matmuls
                    # cover the previous tile's final flush, so no tile
                    # drains with an exp-wait bubble.  pend entries carry
                    # their own (ps_o, ps_sum, head, is-last-pair) context.
                    pend = []

                    def flush():
                        # same-accumulation-group matmuls must be adjacent:
                        # a LDWEIGHTS that follows an accumulating matmul
                        # whose group is being suspended stalls ~95ns on hw
                        # (after a STOPPED group it is free).  Flush up to
                        # TWO pairs at once, all o-matmuls in one burst
                        # then all sum-matmuls.  Non-diag pairs carry a
                        # VE-pre-added pair-sum tile, so their softmax
                        # denominator needs ONE ones-matmul, not two.
                        take, pend[:] = pend[:2], pend[2:]
                        for pt_, sws, ps_o_, ps_sum_, hh_, lastp, _ in take:
                            for k, (s_, w_) in enumerate(sws):
                                nc.tensor.matmul(
                                    ps_o_[:, w_], vt[hh_][:, s_, :],
                                    pt_[:, k, w_], start=(s_ == 0),
                                    stop=(lastp and k == len(sws) - 1))
                        i = 0
                        while i < len(take):
                            pt_, sws, ps_o_, ps_sum_, hh_, lastp, pts = \
                                take[i]
                            if (pts is not None and i + 1 < len(take)
                                    and take[i + 1][6] is not None
                                    and take[i + 1][3] is ps_sum_):
                                # two non-diag pairs of the same t-tile:
                                # VE-add their pair-sums, one ones-matmul
                                ptq = ptsp.tile([P, TA], bf16, tag="ptsum")
                                nc.vector.tensor_add(ptq[:], pts[:],
                                                     take[i + 1][6][:])
                                nc.tensor.matmul(
                                    ps_sum_[:, :], ones_col[:], ptq[:],
                                    start=(sws[0][0] == 0), stop=False)
                                i += 2
                                continue
                            if pts is not None:
                                nc.tensor.matmul(
                                    ps_sum_[:, :], ones_col[:], pts[:],
                                    start=(sws[0][0] == 0), stop=False)
                                i += 1
                                continue
                            for k, (s_, w_) in enumerate(sws):
                                nc.tensor.matmul(
                                    ps_sum_[:, w_], ones_col[:],
                                    pt_[:, k, w_], start=(s_ == 0),
                                    stop=(lastp and k == len(sws) - 1))
                            i += 1

                    for hh in range(HPC):
                        for ta in range(NTA):
                            spt = TA // P
                            tsl = slice(ta * TA, (ta + 1) * TA)
                            ps_o = pso.tile([P, TA], f32, tag="o")
                            ps_sum = pssum.tile([1, TA], f32, tag="sum")
                            nblk = (ta + 1) * spt

                            for pi in range(nblk // 2):
                                ps_sc = pssc.tile([P, 2, TA], f32, tag="sc")
                                pt = ptp.tile([P, 2, TA], bf16, tag="pt")
                                sws = []
                                for k in range(2):
                                    s = 2 * pi + k
                                    diag = s >= ta * spt
                                    t_lo = (s - ta * spt) * P if diag else 0
                                    w = slice(t_lo, TA)
                                    qsl = slice(ta * TA + t_lo,
                                                (ta + 1) * TA)
                                    nc.tensor.matmul(
                                        ps_sc[:, k, w],
                                        kT[hh][:, s * P:(s + 1) * P],
                                        qT[hh][:, qsl],
                                        start=True, stop=True)
                                    sws.append((s, w))
                                # one exp for both chunks; cols outside a
                                # diag chunk's window hold stale psum ->
                                # garbage pt that no matmul reads
                                nc.scalar.activation(pt[:, :, :],
                                                     ps_sc[:, :, :],
                                                     EXP, scale=SCALE)
                                for k, (s, w) in enumerate(sws):
                                    if s >= ta * spt:  # mask the triangle
                                        t_lo = (s - ta * spt) * P
                                        nc.vector.tensor_mul(
                                            pt[:, k, t_lo:t_lo + P],
                                            pt[:, k, t_lo:t_lo + P],
                                            mask_sb[:])
                                pts = None
                                if pi < 2 * ta:  # non-diag pair
                                    pts = ptsp.tile([P, TA], bf16,
                                                    tag="ptsum")
                                    nc.vector.tensor_add(pts[:],
                                                         pt[:, 0, :],
                                                         pt[:, 1, :])
                                pend.append((pt, sws, ps_o, ps_sum, hh,
                                             pi == nblk // 2 - 1, pts))
                                if len(pend) > 2:
                                    flush()
                                if pi == 0 and norm_q:
                                    norm_q.pop(0)()

                            # normalization, deferred: recip on VE,
                            # partition-broadcast on gpsimd, flat multiply
                            # on VE -- issued inside the next tile's block
                            # loop so the PE/Act pipeline never waits.
                            def normalize(ps_o=ps_o, ps_sum=ps_sum,
                                          hh=hh, tsl=tsl):
                                recf = sp.tile([1, TA], f32, tag="recf")
                                nc.vector.reciprocal_approx_fast(
                                    recf[:], ps_sum[:])
                                recb = sp.tile([1, TA], bf16, tag="recb")
                                nc.vector.tensor_copy(recb[:], recf[:])
                                bcb = sp.tile([P, TA], bf16, tag="bcb")
                                nc.gpsimd.partition_broadcast(bcb[:],
                                                              recb[:],
                                                              channels=P)
                                nc.vector.tensor_mul(oT[b][hh][:, tsl],
                                                     ps_o[:], bcb[:])
                            norm_q.append(normalize)

                            if b == 0 and hh == 0:
                                # prefetch all of wout during head 0's
                                # attention (the first out-proj chain needs
                                # every j block)
                                for _ in range(4):
                                    nc.sync.dma_start(
                                        wout_sb[:, nwo, :], woutT.ap()[nwo])
                                    nwo += 1

                    while pend:
                        flush()
                    for t_ in norm_q:
                        t_()

                # ---------------- output projection (pure PE) ----------
                with tc.tile_pool(name=f"psC{b}", bufs=2, space="PSUM") as psc:
                    psc_pool[0] = psc
                    for hh in range(HPC):
                        for cpi in range(NCP):
                            outproj_chain(b, hh, cpi)

    nc.compile()
    return nc


_NC = None


def _get_nc():
    global _NC
    if _NC is None:
        _NC = _build()
    return _NC


def _host_tables():
    pos = np.arange(T, dtype=np.float32)[:, None]
    div = np.exp(np.arange(0, 2 * HALF, 2, dtype=np.float32)
                 * np.float32(-math.log(ROPE_BASE) / (2 * HALF)))
    ang = pos * div[None, :]
    cosv = np.cos(ang).astype(np.float32)   # [T, HALF]
    sinv = np.sin(ang).astype(np.float32)
    cosT = np.ascontiguousarray(cosv.T)     # [HALF, T]
    sinT = np.ascontiguousarray(sinv.T)
    cs2 = np.ascontiguousarray(
        np.concatenate([cosT, cosT], axis=0)).astype(ml_dtypes.bfloat16)
    sn1 = np.ascontiguousarray(sinT).astype(ml_dtypes.bfloat16)
    # triangle mask M[s, w] = 1 iff s <= w
    ww = np.arange(P)[None, :]
    ss = np.arange(P)[:, None]
    maskM = (ss <= ww).astype(ml_dtypes.bfloat16)
    return cs2, sn1, maskM


def _make_in_maps(x, Wqkv, Wout):
    x = np.asarray(x, dtype=np.float32)
    Wqkv = np.asarray(Wqkv, dtype=np.float32)
    Wout = np.asarray(Wout, dtype=np.float32)
    assert x.shape == (B, T, C) and Wqkv.shape == (C, 3 * C) \
        and Wout.shape == (C, C)

    cs2, sn1, maskM = _host_tables()
    # xTt[b, ti, p, ko, u] = x[b, ti*TQ+u, ko*128+p]
    xTt = np.ascontiguousarray(
        x.reshape(B, NT, TQ, KO, P).transpose(0, 1, 4, 3, 2)
    ).astype(ml_dtypes.bfloat16)
    woutT = np.ascontiguousarray(
        Wout.reshape(KO, P, C)).astype(ml_dtypes.bfloat16)

    in_maps = []
    for core in range(NCORES):
        h0 = core * HPC
        cols = slice(h0 * D, (h0 + HPC) * D)
        ws = []
        for part in range(3):
            w = Wqkv[:, part * C:(part + 1) * C][:, cols]  # [C, HPC*D]
            ws.append(np.ascontiguousarray(
                w.reshape(KO, P, HPC * D).transpose(1, 0, 2)
            ).astype(ml_dtypes.bfloat16))
        in_maps.append({
            "xTt": xTt,
            "wq": ws[0], "wk": ws[1], "wv": ws[2],
            "woutT": woutT,
            "cs2": cs2, "sn1": sn1, "maskM": maskM,
        })
    return in_maps


def _run(x, Wqkv, Wout, trace=False):
    nc = _get_nc()
    in_maps = _make_in_maps(x, Wqkv, Wout)
    res = run_bass_kernel_spmd(nc, in_maps, core_ids=list(range(NCORES)),
                               trace=trace)
    out = np.empty((B, T, C), dtype=np.float32)
    for core in range(NCORES):
        out[:, core * HPC * D:(core + 1) * HPC * D, :] = \
            res.results[core]["y"]
    return out, res


def kernel(x, Wqkv, Wout):
    out, _ = _run(x, Wqkv, Wout)
    return out


# revision 49
# speedup vs baseline: 1.0447x; 1.0060x over previous
"""Trainium2 Bass kernel for nn_MultiHeadAttention_63015760167496.

Computation (see reference): qkv = x @ Wqkv; RoPE on q,k; causal softmax
attention per head; out = einsum('bhts,bshd->bhtd', probs, v);
out.reshape(B,T,C) @ Wout  -- the reshape is a *head-major* flatten of
[B,H,T,D] into [B,T,C], so final-output row r = h*128 + t//16 depends only
on head h.  Sharding: head-parallel over 8 cores (2 heads/core); every core
computes its two heads end-to-end and produces final-output rows
[256*i, 256*i+256).  Host concatenates -- no collectives.

All on-device data is bf16 (PSUM accumulation f32), which halves DMA/SBUF
vs f32r at the same 1 cycle/row PE rate.  Attention runs in S^T layout
([s,t]): softmax denominator via a ones-column matmul (partition reduction
on the PE), normalization broadcast via gpsimd.partition_broadcast and a
flat VE multiply, both issued *deferred* (inside the next t-tile's block
loop) so they never gate the PE.  The attention inner loop is
software-pipelined: score blocks are processed in pairs sharing one
two-bank PSUM tile and a single exp instruction, and the PV/sum matmuls of
the previous two pairs are flushed in same-accumulation-group bursts (a
LDWEIGHTS after an accumulating matmul whose group is suspended stalls
~95ns on hw).  O^T is stored flat [d, t]; the out-projection reads it
through a strided LDWEIGHTS view.  Wout is prefetched into SBUF during
attention b=0 so the out-projection phases are pure PE.
"""

import math
import sys

for _p in ("/opt/trn_rl_repo", "/root/.axon_site/_ro/trn_rl_repo"):
    if _p not in sys.path:
        sys.path.insert(0, _p)

import numpy as np
import ml_dtypes

import concourse.bass as bass
import concourse.mybir as mybir
import concourse.tile as tile
from concourse import bacc
from concourse.bass_utils import run_bass_kernel_spmd

B, T, C = 2, 2048, 2048
H = 16            # heads total
D = C // H        # 128 head dim
HALF = D // 2     # 64
P = 128
KO = C // P       # 16 contraction chunks
NCORES = 8
HPC = H // NCORES  # 2 heads per core
TQ = 512          # t-tile for qkv projection
NT = T // TQ      # 4
TA = 512          # t-tile for attention
NTA = T // TA     # 4
NSC = T // P      # 16 s-chunks
ROPE_BASE = 10000.0
SCALE = 1.0 / math.sqrt(D)
TC_ = 512         # col-tile for out projection
NCP = C // TC_    # 4
LOOKAHEAD = 2     # attention software-pipeline depth

f32 = mybir.dt.float32
bf16 = mybir.dt.bfloat16
EXP = mybir.ActivationFunctionType.Exp


def _build():
    nc = bacc.Bacc("TRN2", target_bir_lowering=False, debug=False,
                   num_devices=NCORES)

    # host-pre-tiled x^T: xTt[b, ti, p, ko, u] = x[b, ti*TQ+u, ko*128+p]
    xTt = nc.dram_tensor("xTt", [B, NT, P, KO, TQ], bf16, kind="ExternalInput")
    # host-pre-chunked weights: w[p, ko, m] = W[ko*128+p, m]
    wq = nc.dram_tensor("wq", [P, KO, HPC * D], bf16, kind="ExternalInput")
    wk = nc.dram_tensor("wk", [P, KO, HPC * D], bf16, kind="ExternalInput")
    wv = nc.dram_tensor("wv", [P, KO, HPC * D], bf16, kind="ExternalInput")
    # woutT[j, p, c] = Wout[j*128+p, c]
    woutT = nc.dram_tensor("woutT", [KO, P, C], bf16, kind="ExternalInput")
    cs2 = nc.dram_tensor("cs2", [P, T], bf16, kind="ExternalInput")  # [cos;cos]
    sn1 = nc.dram_tensor("sn1", [HALF, T], bf16, kind="ExternalInput")  # sin
    maskM = nc.dram_tensor("maskM", [P, P], bf16, kind="ExternalInput")
    y = nc.dram_tensor("y", [B, HPC * D, C], f32, kind="ExternalOutput")

    with tile.TileContext(nc) as tc:
        with tc.tile_pool(name="const", bufs=1) as cp_, \
             tc.tile_pool(name="wo", bufs=1) as wop, \
             tc.tile_pool(name="qkv", bufs=1) as qp, \
             tc.tile_pool(name="ot", bufs=1) as op_, \
             tc.tile_pool(name="ys", bufs=4) as yp, \
             tc.tile_pool(name="small", bufs=2) as sp:

            wq_sb = cp_.tile([P, KO, HPC * D], bf16, tag="wq")
            wk_sb = cp_.tile([P, KO, HPC * D], bf16, tag="wk")
            wv_sb = cp_.tile([P, KO, HPC * D], bf16, tag="wv")
            cs_sb = cp_.tile([P, T], bf16, tag="cs")
            sn_sb = cp_.tile([HALF, T], bf16, tag="sn")
            mask_sb = cp_.tile([P, P], bf16, tag="mask")
            wout_sb = wop.tile([P, KO, C], bf16, tag="wout")

            # startup DMAs: wq first (chunked) so the first chain starts
            # ASAP; the first chunk goes through the gpsimd DGE (its
            # framework preamble ends ~2.5us before the sync engine's).
            nc.gpsimd.dma_start(wq_sb[:, 0:8, :], wq.ap()[:, 0:8, :])
            nc.sync.dma_start(wq_sb[:, 8:16, :], wq.ap()[:, 8:16, :])

            ones_f32 = cp_.tile([P, 1], f32, tag="ones_f32")
            nc.vector.memset(ones_f32[:], 1.0)
            ones_col = cp_.tile([P, 1], bf16, tag="ones_col")
            nc.vector.tensor_copy(ones_col[:], ones_f32[:])
            # act-table warmup: force the Exp table load at t=0 instead of
            # in the middle of the first attention block.
            warm_in = cp_.tile([1, 8], f32, tag="warm_in")
            nc.vector.memset(warm_in[:], 0.0)
            warm_out = cp_.tile([1, 8], f32, tag="warm_out")
            nc.scalar.activation(warm_out[:], warm_in[:], EXP, scale=1.0)

            # persistent attention outputs O^T per (b, local head): [d, t]
            oT = [[op_.tile([P, T], bf16, tag=f"oT{b}{hh}", name=f"oT{b}{hh}")
                   for hh in range(HPC)] for b in range(B)]

            def outproj_chain(b, hh, cpi):
                csl = slice(cpi * TC_, (cpi + 1) * TC_)
                psy = psc_pool[0].tile([P, TC_], f32, tag="y")
                # stationary: oT columns {t : t%16 == j}, strided view
                ovw = oT[b][hh].rearrange("p (u j) -> p j u", j=KO)
                for j in range(KO):
                    nc.tensor.matmul(psy[:], ovw[:, j, :],
                                     wout_sb[:, j, csl],
                                     start=(j == 0), stop=(j == KO - 1))
                ysb = yp.tile([P, TC_], f32, tag="ysb")
                nc.scalar.copy(ysb[:], psy[:])
                nc.sync.dma_start(
                    y.ap()[b, hh * D:(hh + 1) * D, csl], ysb[:])

            psc_pool = [None]

            for b in range(B):
                qT = [qp.tile([P, T], bf16, tag=f"qT{hh}", name=f"qT{b}{hh}")
                      for hh in range(HPC)]
                kT = [qp.tile([P, T], bf16, tag=f"kT{hh}", name=f"kT{b}{hh}")
                      for hh in range(HPC)]
                vt = [qp.tile([P, NSC, D], bf16, tag=f"v{hh}", name=f"v{b}{hh}")
                      for hh in range(HPC)]

                # ---------------- QKV projection + RoPE ----------------
                with tc.tile_pool(name=f"xt{b}", bufs=3) as xp, \
                     tc.tile_pool(name=f"psA{b}", bufs=3, space="PSUM") as psa, \
                     tc.tile_pool(name=f"psV{b}", bufs=2, space="PSUM") as psv_p, \
                     tc.tile_pool(name=f"rope{b}", bufs=3) as rp:

                    def qkmm(xt, w_sb, hh, nm):
                        hsl = slice(hh * D, (hh + 1) * D)
                        ps = psa.tile([P, TQ], f32, tag="acc", name=nm)
                        for ko in range(KO):
                            nc.tensor.matmul(ps[:], w_sb[:, ko, hsl],
                                             xt[:, ko, :],
                                             start=(ko == 0),
                                             stop=(ko == KO - 1))
                        return ps

                    def rope(ps, dst, sl):
                        # tcos = ps * [cos;cos]; tsw pre-swaps halves:
                        # tsw[0:64]=q2*sin, tsw[64:128]=q1*sin so the add/sub
                        # reads align on base partitions.  All elementwise
                        # work on the VE (bf16 operands get 2x mode).
                        cs = cs_sb[:, sl]
                        sn = sn_sb[:, sl]
                        tcos = rp.tile([P, TQ], bf16, tag="tcos")
                        tsw = rp.tile([P, TQ], bf16, tag="tsw")
                        nc.vector.tensor_mul(tcos[:], ps[:], cs)
                        nc.vector.tensor_mul(tsw[0:HALF, :], ps[HALF:P, :], sn)
                        nc.vector.tensor_mul(tsw[HALF:P, :], ps[0:HALF, :], sn)
                        nc.vector.tensor_sub(dst[0:HALF, sl],
                                             tcos[0:HALF, :], tsw[0:HALF, :])
                        nc.vector.tensor_add(dst[HALF:P, sl],
                                             tcos[HALF:P, :], tsw[HALF:P, :])

                    def vchain(xt, ti):
                        for sub in range(TQ // P):
                            psv = psv_p.tile([P, HPC * D], f32, tag="acc")
                            for ko in range(KO):
                                nc.tensor.matmul(
                                    psv[:], xt[:, ko, sub * P:(sub + 1) * P],
                                    wv_sb[:, ko, :],
                                    start=(ko == 0), stop=(ko == KO - 1))
                            tci = ti * (TQ // P) + sub
                            for hh in range(HPC):
                                # Act engine is idle during QKV; it does the
                                # psum->sbuf v copies.
                                nc.scalar.copy(
                                    vt[hh][:, tci, :],
                                    psv[:, hh * D:(hh + 1) * D])

                    xts = {}
                    for ti in range(NT):
                        xts[ti] = xp.tile([P, KO, TQ], bf16, tag="xt",
                                          name=f"xt{b}_{ti}")

                    if b == 0:
                        # Startup is a DMA-bandwidth wall: ~7MB must land in
                        # the first ~30us.  Chunk the first two x tiles so
                        # chains pace behind arriving data, interleave wq/x
                        # chunks in ko-consumption order, and defer ti0's
                        # v-chains until after ti1's q/k so wv is needed
                        # later.
                        for g in range(4):
                            nc.sync.dma_start(
                                xts[0][:, 4 * g:4 * g + 4, :],
                                xTt.ap()[b, 0, :, 4 * g:4 * g + 4, :])
                        ps = qkmm(xts[0], wq_sb, 0, "acc0_q0")
                        nc.sync.dma_start(wk_sb[:], wk.ap())
                        nc.sync.dma_start(cs_sb[:], cs2.ap())
                        nc.sync.dma_start(sn_sb[:], sn1.ap())
                        rope(ps, qT[0], slice(0, TQ))
                        rope(qkmm(xts[0], wq_sb, 1, "acc0_q1"), qT[1],
                             slice(0, TQ))
                        for g in range(4):
                            nc.sync.dma_start(
                                xts[1][:, 4 * g:4 * g + 4, :],
                                xTt.ap()[b, 1, :, 4 * g:4 * g + 4, :])
                        rope(qkmm(xts[0], wk_sb, 0, "acc0_k0"), kT[0],
                             slice(0, TQ))
                        nc.sync.dma_start(wv_sb[:], wv.ap())
                        nc.sync.dma_start(mask_sb[:], maskM.ap())
                        rope(qkmm(xts[0], wk_sb, 1, "acc0_k1"), kT[1],
                             slice(0, TQ))
                        sl1 = slice(TQ, 2 * TQ)
                        rope(qkmm(xts[1], wq_sb, 0, "acc1_q0"), qT[0], sl1)
                        rope(qkmm(xts[1], wq_sb, 1, "acc1_q1"), qT[1], sl1)
                        rope(qkmm(xts[1], wk_sb, 0, "acc1_k0"), kT[0], sl1)
                        rope(qkmm(xts[1], wk_sb, 1, "acc1_k1"), kT[1], sl1)
                        vchain(xts[1], 1)
                        vchain(xts[0], 0)
                        rest = range(2, NT)
                    else:
                        rest = range(NT)

                    for ti in rest:
                        sl = slice(ti * TQ, (ti + 1) * TQ)
                        xt = xts[ti]
                        nc.sync.dma_start(xt[:], xTt.ap()[b, ti])
                        for hh in range(HPC):
                            rope(qkmm(xt, wq_sb, hh, f"a{ti}q{hh}"),
                                 qT[hh], sl)
                            rope(qkmm(xt, wk_sb, hh, f"a{ti}k{hh}"),
                                 kT[hh], sl)
                        vchain(xt, ti)

                # ------------- attention (S^T layout) + interleaved -----
                # ------------- out-projection of the previous head ------
                # s-chunks are processed in PAIRS sharing one 2-bank PSUM
                # tile and a single exp instruction, so the Act engine
                # (1024 cols + one fixed overhead) runs faster than the
                # PE's 6 matmuls per pair and never paces the pipeline.
                with tc.tile_pool(name=f"psBsc{b}", bufs=2, space="PSUM") as pssc, \
                     tc.tile_pool(name=f"psBo{b}", bufs=2, space="PSUM") as pso, \
                     tc.tile_pool(name=f"psBsum{b}", bufs=2, space="PSUM") as pssum, \
                     tc.tile_pool(name=f"pt{b}", bufs=4) as ptp, \
                     tc.tile_pool(name=f"pts{b}", bufs=4) as ptsp:
                    nwo = 0   # wout prefetch cursor (b == 0 only)
                    # Deferred-issue queue: each t-tile's normalize is
                    # issued inside the NEXT tile's block loop (ps_o and
                    # ps_sum have bufs=2, so it must be issued before the
                    # slot cycles) -- the PE/Act pipeline never waits on it.
                    norm_q = []
                    # the score->PV pipeline is carried ACROSS (head,
                    # t-tile) boundaries: the next tile's score matmuls
                    # cover the previous tile's final flush, so no tile
                    # drains with an exp-wait bubble.  pend entries carry
                    # their own (ps_o, ps_sum, head, is-last-pair) context.
                    pend = []

                    def flush():
                        # same-accumulation-group matmuls must be adjacent:
                        # a LDWEIGHTS that follows an accumulating matmul
                        # whose group is being suspended stalls ~95ns on hw
                        # (after a STOPPED group it is free).  Flush up to
                        # TWO pairs at once, all o-matmuls in one burst
                        # then all sum-matmuls.  Non-diag pairs carry a
                        # VE-pre-added pair-sum tile, so their softmax
                        # denominator needs ONE ones-matmul, not two.
                        take, pend[:] = pend[:2], pend[2:]
                        for (pt_, sws, ps_o_, ps_sum_, hh_, lastp, pts,
                             ptq) in take:
                            for k, (s_, w_) in enumerate(sws):
                                nc.tensor.matmul(
                                    ps_o_[:, w_], vt[hh_][:, s_, :],
                                    pt_[:, k, w_], start=(s_ == 0),
                                    stop=(lastp and k == len(sws) - 1))
                        # flush pops are (even, odd)-pair aligned within a
                        # tile (every tile has an even pair count), so a
                        # quad-sum precomputed on the odd entry (ready two
                        # pairs early, no VE wait) covers the whole take
                        # with ONE ones-matmul.
                        if (len(take) == 2 and take[1][7] is not None
                                and take[1][3] is take[0][3]):
                            nc.tensor.matmul(
                                take[0][3][:, :], ones_col[:],
                                take[1][7][:],
                                start=(take[0][1][0][0] == 0), stop=False)
                            return
                        for (pt_, sws, ps_o_, ps_sum_, hh_, lastp, pts,
                             ptq) in take:
                            if pts is not None:
                                nc.tensor.matmul(
                                    ps_sum_[:, :], ones_col[:], pts[:],
                                    start=(sws[0][0] == 0), stop=False)
                                continue
                            for k, (s_, w_) in enumerate(sws):
                                nc.tensor.matmul(
                                    ps_sum_[:, w_], ones_col[:],
                                    pt_[:, k, w_], start=(s_ == 0),
                                    stop=(lastp and k == len(sws) - 1))

                    for hh in range(HPC):
                        for ta in range(NTA):
                            spt = TA // P
                            tsl = slice(ta * TA, (ta + 1) * TA)
                            ps_o = pso.tile([P, TA], f32, tag="o")
                            ps_sum = pssum.tile([1, TA], f32, tag="sum")
                            nblk = (ta + 1) * spt
                            prev_pts = None

                            for pi in range(nblk // 2):
                                ps_sc = pssc.tile([P, 2, TA], f32, tag="sc")
                                pt = ptp.tile([P, 2, TA], bf16, tag="pt")
                                sws = []
                                for k in range(2):
                                    s = 2 * pi + k
                                    diag = s >= ta * spt
                                    t_lo = (s - ta * spt) * P if diag else 0
                                    w = slice(t_lo, TA)
                                    qsl = slice(ta * TA + t_lo,
                                                (ta + 1) * TA)
                                    nc.tensor.matmul(
                                        ps_sc[:, k, w],
                                        kT[hh][:, s * P:(s + 1) * P],
                                        qT[hh][:, qsl],
                                        start=True, stop=True)
                                    sws.append((s, w))
                                # one exp for both chunks; cols outside a
                                # diag chunk's window hold stale psum ->
                                # garbage pt that no matmul reads
                                nc.scalar.activation(pt[:, :, :],
                                                     ps_sc[:, :, :],
                                                     EXP, scale=SCALE)
                                for k, (s, w) in enumerate(sws):
                                    if s >= ta * spt:  # mask the triangle
                                        t_lo = (s - ta * spt) * P
                                        nc.vector.tensor_mul(
                                            pt[:, k, t_lo:t_lo + P],
                                            pt[:, k, t_lo:t_lo + P],
                                            mask_sb[:])
                                pts = None
                                ptq = None
                                if pi < 2 * ta:  # non-diag pair
                                    pts = ptsp.tile([P, TA], bf16,
                                                    tag="ptsum")
                                    nc.vector.tensor_add(pts[:],
                                                         pt[:, 0, :],
                                                         pt[:, 1, :])
                                    if pi % 2 == 1:
                                        # quad-sum of pairs (pi-1, pi),
                                        # ready well before its flush
                                        ptq = ptsp.tile([P, TA], bf16,
                                                        tag="ptsum")
                                        nc.vector.tensor_add(
                                            ptq[:], prev_pts[:], pts[:])
                                prev_pts = pts
                                pend.append((pt, sws, ps_o, ps_sum, hh,
                                             pi == nblk // 2 - 1, pts,
                                             ptq))
                                if len(pend) > 2:
                                    flush()
                                if pi == 0 and norm_q:
                                    norm_q.pop(0)()

                            # normalization, deferred: recip on VE,
                            # partition-broadcast on gpsimd, flat multiply
                            # on VE -- issued inside the next tile's block
                            # loop so the PE/Act pipeline never waits.
                            def normalize(ps_o=ps_o, ps_sum=ps_sum,
                                          hh=hh, tsl=tsl):
                                recf = sp.tile([1, TA], f32, tag="recf")
                                nc.vector.reciprocal_approx_fast(
                                    recf[:], ps_sum[:])
                                recb = sp.tile([1, TA], bf16, tag="recb")
                                nc.vector.tensor_copy(recb[:], recf[:])
                                bcb = sp.tile([P, TA], bf16, tag="bcb")
                                nc.gpsimd.partition_broadcast(bcb[:],
                                                              recb[:],
                                                              channels=P)
                                nc.vector.tensor_mul(oT[b][hh][:, tsl],
                                                     ps_o[:], bcb[:])
                            norm_q.append(normalize)

                            if b == 0 and hh == 0:
                                # prefetch all of wout during head 0's
                                # attention (the first out-proj chain needs
                                # every j block)
                                for _ in range(4):
                                    nc.sync.dma_start(
                                        wout_sb[:, nwo, :], woutT.ap()[nwo])
                                    nwo += 1

                    while pend:
                        flush()
                    for t_ in norm_q:
                        t_()

                # ---------------- output projection (pure PE) ----------
                with tc.tile_pool(name=f"psC{b}", bufs=2, space="PSUM") as psc:
                    psc_pool[0] = psc
                    for hh in range(HPC):
                        for cpi in range(NCP):
                            outproj_chain(b, hh, cpi)

    nc.compile()
    return nc


_NC = None


def _get_nc():
    global _NC
    if _NC is None:
        _NC = _build()
    return _NC


def _host_tables():
    pos = np.arange(T, dtype=np.float32)[:, None]
    div = np.exp(np.arange(0, 2 * HALF, 2, dtype=np.float32)
                 * np.float32(-math.log(ROPE_BASE) / (2 * HALF)))
    ang = pos * div[None, :]
    cosv = np.cos(ang).astype(np.float32)   # [T, HALF]
    sinv = np.sin(ang).astype(np.float32)
    cosT = np.ascontiguousarray(cosv.T)     # [HALF, T]
    sinT = np.ascontiguousarray(sinv.T)
    cs2 = np.ascontiguousarray(
        np.concatenate([cosT, cosT], axis=0)).astype(ml_dtypes.bfloat16)
    sn1 = np.ascontiguousarray(sinT).astype(ml_dtypes.bfloat16)
    # triangle mask M[s, w] = 1 iff s <= w
    ww = np.arange(P)[None, :]
    ss = np.arange(P)[:, None]
    maskM = (ss <= ww).astype(ml_dtypes.bfloat16)
    return cs2, sn1, maskM


def _make_in_maps(x, Wqkv, Wout):
    x = np.asarray(x, dtype=np.float32)
    Wqkv = np.asarray(Wqkv, dtype=np.float32)
    Wout = np.asarray(Wout, dtype=np.float32)
    assert x.shape == (B, T, C) and Wqkv.shape == (C, 3 * C) \
        and Wout.shape == (C, C)

    cs2, sn1, maskM = _host_tables()
    # xTt[b, ti, p, ko, u] = x[b, ti*TQ+u, ko*128+p]
    xTt = np.ascontiguousarray(
        x.reshape(B, NT, TQ, KO, P).transpose(0, 1, 4, 3, 2)
    ).astype(ml_dtypes.bfloat16)
    woutT = np.ascontiguousarray(
        Wout.reshape(KO, P, C)).astype(ml_dtypes.bfloat16)

    in_maps = []
    for core in range(NCORES):
        h0 = core * HPC
        cols = slice(h0 * D, (h0 + HPC) * D)
        ws = []
        for part in range(3):
            w = Wqkv[:, part * C:(part + 1) * C][:, cols]  # [C, HPC*D]
            ws.append(np.ascontiguousarray(
                w.reshape(KO, P, HPC * D).transpose(1, 0, 2)
            ).astype(ml_dtypes.bfloat16))
        in_maps.append({
            "xTt": xTt,
            "wq": ws[0], "wk": ws[1], "wv": ws[2],
            "woutT": woutT,
            "cs2": cs2, "sn1": sn1, "maskM": maskM,
        })
    return in_maps


def _run(x, Wqkv, Wout, trace=False):
    nc = _get_nc()
    in_maps = _make_in_maps(x, Wqkv, Wout)
    res = run_bass_kernel_spmd(nc, in_maps, core_ids=list(range(NCORES)),
                               trace=trace)
    out = np.empty((B, T, C), dtype=np.float32)
    for core in range(NCORES):
        out[:, core * HPC * D:(core + 1) * HPC * D, :] = \
            res.results[core]["y"]
    return out, res


def kernel(x, Wqkv, Wout):
    out, _ = _run(x, Wqkv, Wout)
    return out


# revision 51
# speedup vs baseline: 1.0567x; 1.0115x over previous
"""Trainium2 Bass kernel for nn_MultiHeadAttention_63015760167496.

Computation (see reference): qkv = x @ Wqkv; RoPE on q,k; causal softmax
attention per head; out = einsum('bhts,bshd->bhtd', probs, v);
out.reshape(B,T,C) @ Wout  -- the reshape is a *head-major* flatten of
[B,H,T,D] into [B,T,C], so final-output row r = h*128 + t//16 depends only
on head h.  Sharding: head-parallel over 8 cores (2 heads/core); every core
computes its two heads end-to-end and produces final-output rows
[256*i, 256*i+256).  Host concatenates -- no collectives.

All on-device data is bf16 (PSUM accumulation f32), which halves DMA/SBUF
vs f32r at the same 1 cycle/row PE rate.  Attention runs in S^T layout
([s,t]): softmax denominator via a ones-column matmul (partition reduction
on the PE), normalization broadcast via gpsimd.partition_broadcast and a
flat VE multiply, both issued *deferred* (inside the next t-tile's block
loop) so they never gate the PE.  The attention inner loop is
software-pipelined: score blocks are processed in pairs sharing one
two-bank PSUM tile and a single exp instruction, and the PV/sum matmuls of
the previous two pairs are flushed in same-accumulation-group bursts (a
LDWEIGHTS after an accumulating matmul whose group is suspended stalls
~95ns on hw).  O^T is stored flat [d, t]; the out-projection reads it
through a strided LDWEIGHTS view.  Wout is prefetched into SBUF during
attention b=0 so the out-projection phases are pure PE.
"""

import math
import sys

for _p in ("/opt/trn_rl_repo", "/root/.axon_site/_ro/trn_rl_repo"):
    if _p not in sys.path:
        sys.path.insert(0, _p)

import numpy as np
import ml_dtypes

import concourse.bass as bass
import concourse.mybir as mybir
import concourse.tile as tile
from concourse import bacc
from concourse.bass_utils import run_bass_kernel_spmd

B, T, C = 2, 2048, 2048
H = 16            # heads total
D = C // H        # 128 head dim
HALF = D // 2     # 64
P = 128
KO = C // P       # 16 contraction chunks
NCORES = 8
HPC = H // NCORES  # 2 heads per core
TQ = 512          # t-tile for qkv projection
NT = T // TQ      # 4
TA = 512          # t-tile for attention
NTA = T // TA     # 4
NSC = T // P      # 16 s-chunks
ROPE_BASE = 10000.0
SCALE = 1.0 / math.sqrt(D)
TC_ = 512         # col-tile for out projection
NCP = C // TC_    # 4
LOOKAHEAD = 2     # attention software-pipeline depth

f32 = mybir.dt.float32
bf16 = mybir.dt.bfloat16
EXP = mybir.ActivationFunctionType.Exp


def _build():
    nc = bacc.Bacc("TRN2", target_bir_lowering=False, debug=False,
                   num_devices=NCORES)

    # host-pre-tiled x^T: xTt[b, ti, p, ko, u] = x[b, ti*TQ+u, ko*128+p]
    xTt = nc.dram_tensor("xTt", [B, NT, P, KO, TQ], bf16, kind="ExternalInput")
    # host-pre-chunked weights: w[p, ko, m] = W[ko*128+p, m]
    wq = nc.dram_tensor("wq", [P, KO, HPC * D], bf16, kind="ExternalInput")
    wk = nc.dram_tensor("wk", [P, KO, HPC * D], bf16, kind="ExternalInput")
    wv = nc.dram_tensor("wv", [P, KO, HPC * D], bf16, kind="ExternalInput")
    # woutT[j, p, c] = Wout[j*128+p, c]
    woutT = nc.dram_tensor("woutT", [KO, P, C], bf16, kind="ExternalInput")
    cs2 = nc.dram_tensor("cs2", [P, T], bf16, kind="ExternalInput")  # [cos;cos]
    sn1 = nc.dram_tensor("sn1", [HALF, T], bf16, kind="ExternalInput")  # sin
    maskM = nc.dram_tensor("maskM", [P, P], bf16, kind="ExternalInput")
    y = nc.dram_tensor("y", [B, HPC * D, C], f32, kind="ExternalOutput")

    with tile.TileContext(nc) as tc:
        with tc.tile_pool(name="const", bufs=1) as cp_, \
             tc.tile_pool(name="wo", bufs=1) as wop, \
             tc.tile_pool(name="qkv", bufs=1) as qp, \
             tc.tile_pool(name="ot", bufs=1) as op_, \
             tc.tile_pool(name="ys", bufs=4) as yp, \
             tc.tile_pool(name="small", bufs=2) as sp:

            wq_sb = cp_.tile([P, KO, HPC * D], bf16, tag="wq")
            wk_sb = cp_.tile([P, KO, HPC * D], bf16, tag="wk")
            wv_sb = cp_.tile([P, KO, HPC * D], bf16, tag="wv")
            cs_sb = cp_.tile([P, T], bf16, tag="cs")
            sn_sb = cp_.tile([HALF, T], bf16, tag="sn")
            mask_sb = cp_.tile([P, P], bf16, tag="mask")
            wout_sb = wop.tile([P, KO, C], bf16, tag="wout")

            # startup DMAs: wq first (chunked) so the first chain starts
            # ASAP; the first chunk goes through the gpsimd DGE (its
            # framework preamble ends ~2.5us before the sync engine's).
            nc.gpsimd.dma_start(wq_sb[:, 0:8, :], wq.ap()[:, 0:8, :])
            nc.sync.dma_start(wq_sb[:, 8:16, :], wq.ap()[:, 8:16, :])

            ones_f32 = cp_.tile([P, 1], f32, tag="ones_f32")
            nc.vector.memset(ones_f32[:], 1.0)
            ones_col = cp_.tile([P, 1], bf16, tag="ones_col")
            nc.vector.tensor_copy(ones_col[:], ones_f32[:])
            # act-table warmup: force the Exp table load at t=0 instead of
            # in the middle of the first attention block.
            warm_in = cp_.tile([1, 8], f32, tag="warm_in")
            nc.vector.memset(warm_in[:], 0.0)
            warm_out = cp_.tile([1, 8], f32, tag="warm_out")
            nc.scalar.activation(warm_out[:], warm_in[:], EXP, scale=1.0)

            # persistent attention outputs O^T per (b, local head): [d, t]
            oT = [[op_.tile([P, T], bf16, tag=f"oT{b}{hh}", name=f"oT{b}{hh}")
                   for hh in range(HPC)] for b in range(B)]

            def outproj_chain(b, hh, cpi):
                csl = slice(cpi * TC_, (cpi + 1) * TC_)
                psy = psc_pool[0].tile([P, TC_], f32, tag="y")
                # stationary: oT columns {t : t%16 == j}, strided view
                ovw = oT[b][hh].rearrange("p (u j) -> p j u", j=KO)
                for j in range(KO):
                    nc.tensor.matmul(psy[:], ovw[:, j, :],
                                     wout_sb[:, j, csl],
                                     start=(j == 0), stop=(j == KO - 1))
                ysb = yp.tile([P, TC_], f32, tag="ysb")
                nc.scalar.copy(ysb[:], psy[:])
                nc.sync.dma_start(
                    y.ap()[b, hh * D:(hh + 1) * D, csl], ysb[:])

            psc_pool = [None]

            for b in range(B):
                qT = [qp.tile([P, T], bf16, tag=f"qT{hh}", name=f"qT{b}{hh}")
                      for hh in range(HPC)]
                kT = [qp.tile([P, T], bf16, tag=f"kT{hh}", name=f"kT{b}{hh}")
                      for hh in range(HPC)]
                vt = [qp.tile([P, NSC, D], bf16, tag=f"v{hh}", name=f"v{b}{hh}")
                      for hh in range(HPC)]

                # ---------------- QKV projection + RoPE ----------------
                with tc.tile_pool(name=f"xt{b}", bufs=3) as xp, \
                     tc.tile_pool(name=f"psA{b}", bufs=3, space="PSUM") as psa, \
                     tc.tile_pool(name=f"psV{b}", bufs=2, space="PSUM") as psv_p, \
                     tc.tile_pool(name=f"rope{b}", bufs=3) as rp:

                    def qkmm(xt, w_sb, hh, nm):
                        hsl = slice(hh * D, (hh + 1) * D)
                        ps = psa.tile([P, TQ], f32, tag="acc", name=nm)
                        for ko in range(KO):
                            nc.tensor.matmul(ps[:], w_sb[:, ko, hsl],
                                             xt[:, ko, :],
                                             start=(ko == 0),
                                             stop=(ko == KO - 1))
                        return ps

                    def rope(ps, dst, sl):
                        # tcos = ps * [cos;cos]; tsw pre-swaps halves:
                        # tsw[0:64]=q2*sin, tsw[64:128]=q1*sin so the add/sub
                        # reads align on base partitions.  All elementwise
                        # work on the VE (bf16 operands get 2x mode).
                        cs = cs_sb[:, sl]
                        sn = sn_sb[:, sl]
                        tcos = rp.tile([P, TQ], bf16, tag="tcos")
                        tsw = rp.tile([P, TQ], bf16, tag="tsw")
                        nc.vector.tensor_mul(tcos[:], ps[:], cs)
                        nc.vector.tensor_mul(tsw[0:HALF, :], ps[HALF:P, :], sn)
                        nc.vector.tensor_mul(tsw[HALF:P, :], ps[0:HALF, :], sn)
                        nc.vector.tensor_sub(dst[0:HALF, sl],
                                             tcos[0:HALF, :], tsw[0:HALF, :])
                        nc.vector.tensor_add(dst[HALF:P, sl],
                                             tcos[HALF:P, :], tsw[HALF:P, :])

                    def vchain(xt, ti):
                        for sub in range(TQ // P):
                            psv = psv_p.tile([P, HPC * D], f32, tag="acc")
                            for ko in range(KO):
                                nc.tensor.matmul(
                                    psv[:], xt[:, ko, sub * P:(sub + 1) * P],
                                    wv_sb[:, ko, :],
                                    start=(ko == 0), stop=(ko == KO - 1))
                            tci = ti * (TQ // P) + sub
                            for hh in range(HPC):
                                # Act engine is idle during QKV; it does the
                                # psum->sbuf v copies.
                                nc.scalar.copy(
                                    vt[hh][:, tci, :],
                                    psv[:, hh * D:(hh + 1) * D])

                    xts = {}
                    for ti in range(NT):
                        xts[ti] = xp.tile([P, KO, TQ], bf16, tag="xt",
                                          name=f"xt{b}_{ti}")

                    if b == 0:
                        # Startup is a DMA-bandwidth wall: ~7MB must land in
                        # the first ~30us.  Chunk the first two x tiles so
                        # chains pace behind arriving data, interleave wq/x
                        # chunks in ko-consumption order, and defer ti0's
                        # v-chains until after ti1's q/k so wv is needed
                        # later.
                        for g in range(4):
                            nc.sync.dma_start(
                                xts[0][:, 4 * g:4 * g + 4, :],
                                xTt.ap()[b, 0, :, 4 * g:4 * g + 4, :])
                        ps = qkmm(xts[0], wq_sb, 0, "acc0_q0")
                        nc.sync.dma_start(wk_sb[:], wk.ap())
                        nc.sync.dma_start(cs_sb[:], cs2.ap())
                        nc.sync.dma_start(sn_sb[:], sn1.ap())
                        rope(ps, qT[0], slice(0, TQ))
                        rope(qkmm(xts[0], wq_sb, 1, "acc0_q1"), qT[1],
                             slice(0, TQ))
                        for g in range(4):
                            nc.sync.dma_start(
                                xts[1][:, 4 * g:4 * g + 4, :],
                                xTt.ap()[b, 1, :, 4 * g:4 * g + 4, :])
                        rope(qkmm(xts[0], wk_sb, 0, "acc0_k0"), kT[0],
                             slice(0, TQ))
                        nc.sync.dma_start(wv_sb[:], wv.ap())
                        nc.sync.dma_start(mask_sb[:], maskM.ap())
                        rope(qkmm(xts[0], wk_sb, 1, "acc0_k1"), kT[1],
                             slice(0, TQ))
                        sl1 = slice(TQ, 2 * TQ)
                        rope(qkmm(xts[1], wq_sb, 0, "acc1_q0"), qT[0], sl1)
                        rope(qkmm(xts[1], wq_sb, 1, "acc1_q1"), qT[1], sl1)
                        rope(qkmm(xts[1], wk_sb, 0, "acc1_k0"), kT[0], sl1)
                        rope(qkmm(xts[1], wk_sb, 1, "acc1_k1"), kT[1], sl1)
                        vchain(xts[1], 1)
                        vchain(xts[0], 0)
                        rest = range(2, NT)
                    else:
                        rest = range(NT)

                    for ti in rest:
                        sl = slice(ti * TQ, (ti + 1) * TQ)
                        xt = xts[ti]
                        nc.sync.dma_start(xt[:], xTt.ap()[b, ti])
                        for hh in range(HPC):
                            rope(qkmm(xt, wq_sb, hh, f"a{ti}q{hh}"),
                                 qT[hh], sl)
                            rope(qkmm(xt, wk_sb, hh, f"a{ti}k{hh}"),
                                 kT[hh], sl)
                        vchain(xt, ti)

                # ------------- attention (S^T layout) + interleaved -----
                # ------------- out-projection of the previous head ------
                # s-chunks are processed in PAIRS sharing one 2-bank PSUM
                # tile and a single exp instruction, so the Act engine
                # (1024 cols + one fixed overhead) runs faster than the
                # PE's 6 matmuls per pair and never paces the pipeline.
                with tc.tile_pool(name=f"psBsc{b}", bufs=2, space="PSUM") as pssc, \
                     tc.tile_pool(name=f"psBo{b}", bufs=2, space="PSUM") as pso, \
                     tc.tile_pool(name=f"psBsum{b}", bufs=2, space="PSUM") as pssum, \
                     tc.tile_pool(name=f"pt{b}", bufs=4) as ptp, \
                     tc.tile_pool(name=f"pts{b}", bufs=4) as ptsp:
                    nwo = 0   # wout prefetch cursor (b == 0 only)
                    # Deferred-issue queue: each t-tile's normalize is
                    # issued inside the NEXT tile's block loop (ps_o and
                    # ps_sum have bufs=2, so it must be issued before the
                    # slot cycles) -- the PE/Act pipeline never waits on it.
                    norm_q = []
                    # the score->PV pipeline is carried ACROSS (head,
                    # t-tile) boundaries: the next tile's score matmuls
                    # cover the previous tile's final flush, so no tile
                    # drains with an exp-wait bubble.  pend entries carry
                    # their own (ps_o, ps_sum, head, is-last-pair) context.
                    pend = []

                    def flush():
                        # same-accumulation-group matmuls must be adjacent:
                        # a LDWEIGHTS that follows an accumulating matmul
                        # whose group is being suspended stalls ~95ns on hw
                        # (after a STOPPED group it is free).  Flush up to
                        # TWO pairs at once, all o-matmuls in one burst
                        # then all sum-matmuls.  Non-diag pairs carry a
                        # VE-pre-added pair-sum tile, so their softmax
                        # denominator needs ONE ones-matmul, not two.
                        take, pend[:] = pend[:2], pend[2:]
                        for (pt_, sws, ps_o_, ps_sum_, hh_, lastp, pts,
                             ptq, dg) in take:
                            for k, (s_, w_) in enumerate(sws):
                                nc.tensor.matmul(
                                    ps_o_[:, w_], vt[hh_][:, s_, :],
                                    pt_[:, k, w_], start=(s_ == 0),
                                    stop=(lastp and k == len(sws) - 1))
                        # flush pops are (even, odd)-pair aligned within a
                        # tile (every tile has an even pair count), so a
                        # quad-sum precomputed on the odd entry (ready two
                        # pairs early, no VE wait) covers the whole take
                        # with ONE ones-matmul.
                        if (len(take) == 2 and take[1][7] is not None
                                and take[1][3] is take[0][3]):
                            nc.tensor.matmul(
                                take[0][3][:, :], ones_col[:],
                                take[1][7][:],
                                start=(take[0][1][0][0] == 0), stop=False)
                            return
                        for (pt_, sws, ps_o_, ps_sum_, hh_, lastp, pts,
                             ptq, dg) in take:
                            first = sws[0][0] == 0
                            if pts is not None:
                                nc.tensor.matmul(
                                    ps_sum_[:, :], ones_col[:], pts[:],
                                    start=first, stop=False)
                                continue
                            if dg is not None and not first:
                                # both accumulate (start would zero the
                                # whole 2KB bank region, clobbering the
                                # other sub-range's partial sums)
                                ptd, lo, ov = dg
                                nc.tensor.matmul(
                                    ps_sum_[:, ov:TA], ones_col[:],
                                    ptd[:, ov:TA],
                                    start=False, stop=False)
                                nc.tensor.matmul(
                                    ps_sum_[:, lo:ov], ones_col[:],
                                    pt_[:, 0, lo:ov],
                                    start=False, stop=lastp)
                                continue
                            for k, (s_, w_) in enumerate(sws):
                                nc.tensor.matmul(
                                    ps_sum_[:, w_], ones_col[:],
                                    pt_[:, k, w_], start=(s_ == 0),
                                    stop=(lastp and k == len(sws) - 1))

                    for hh in range(HPC):
                        for ta in range(NTA):
                            spt = TA // P
                            tsl = slice(ta * TA, (ta + 1) * TA)
                            ps_o = pso.tile([P, TA], f32, tag="o")
                            ps_sum = pssum.tile([1, TA], f32, tag="sum")
                            nblk = (ta + 1) * spt
                            prev_pts = None

                            for pi in range(nblk // 2):
                                ps_sc = pssc.tile([P, 2, TA], f32, tag="sc")
                                pt = ptp.tile([P, 2, TA], bf16, tag="pt")
                                sws = []
                                for k in range(2):
                                    s = 2 * pi + k
                                    diag = s >= ta * spt
                                    t_lo = (s - ta * spt) * P if diag else 0
                                    w = slice(t_lo, TA)
                                    qsl = slice(ta * TA + t_lo,
                                                (ta + 1) * TA)
                                    nc.tensor.matmul(
                                        ps_sc[:, k, w],
                                        kT[hh][:, s * P:(s + 1) * P],
                                        qT[hh][:, qsl],
                                        start=True, stop=True)
                                    sws.append((s, w))
                                # one exp for both chunks; cols outside a
                                # diag chunk's window hold stale psum ->
                                # garbage pt that no matmul reads
                                nc.scalar.activation(pt[:, :, :],
                                                     ps_sc[:, :, :],
                                                     EXP, scale=SCALE)
                                for k, (s, w) in enumerate(sws):
                                    if s >= ta * spt:  # mask the triangle
                                        t_lo = (s - ta * spt) * P
                                        nc.vector.tensor_mul(
                                            pt[:, k, t_lo:t_lo + P],
                                            pt[:, k, t_lo:t_lo + P],
                                            mask_sb[:])
                                pts = None
                                ptq = None
                                dg = None
                                if pi < 2 * ta:  # non-diag pair
                                    pts = ptsp.tile([P, TA], bf16,
                                                    tag="ptsum")
                                    nc.vector.tensor_add(pts[:],
                                                         pt[:, 0, :],
                                                         pt[:, 1, :])
                                    if pi % 2 == 1:
                                        # quad-sum of pairs (pi-1, pi),
                                        # ready well before its flush
                                        ptq = ptsp.tile([P, TA], bf16,
                                                        tag="ptsum")
                                        nc.vector.tensor_add(
                                            ptq[:], prev_pts[:], pts[:])
                                else:
                                    # diag pair: chunks are valid on
                                    # [lo, TA) and [lo+128, TA); VE-add
                                    # the (post-mask) overlap so the sum
                                    # needs one merged ones-matmul plus a
                                    # 128-wide strip from chunk A.
                                    lo = sws[0][1].start
                                    ov = lo + P
                                    ptd = ptsp.tile([P, TA], bf16,
                                                    tag="ptsum")
                                    nc.vector.tensor_add(
                                        ptd[:, ov:TA], pt[:, 0, ov:TA],
                                        pt[:, 1, ov:TA])
                                    dg = (ptd, lo, ov)
                                prev_pts = pts
                                pend.append((pt, sws, ps_o, ps_sum, hh,
                                             pi == nblk // 2 - 1, pts,
                                             ptq, dg))
                                if len(pend) > 2:
                                    flush()
                                if pi == 0 and norm_q:
                                    norm_q.pop(0)()

                            # normalization, deferred: recip on VE,
                            # partition-broadcast on gpsimd, flat multiply
                            # on VE -- issued inside the next tile's block
                            # loop so the PE/Act pipeline never waits.
                            def normalize(ps_o=ps_o, ps_sum=ps_sum,
                                          hh=hh, tsl=tsl):
                                recf = sp.tile([1, TA], f32, tag="recf")
                                nc.vector.reciprocal_approx_fast(
                                    recf[:], ps_sum[:])
                                recb = sp.tile([1, TA], bf16, tag="recb")
                                nc.vector.tensor_copy(recb[:], recf[:])
                                bcb = sp.tile([P, TA], bf16, tag="bcb")
                                nc.gpsimd.partition_broadcast(bcb[:],
                                                              recb[:],
                                                              channels=P)
                                nc.vector.tensor_mul(oT[b][hh][:, tsl],
                                                     ps_o[:], bcb[:])
                            norm_q.append(normalize)

                            if b == 0 and hh == 0:
                                # prefetch all of wout during head 0's
                                # attention (the first out-proj chain needs
                                # every j block)
                                for _ in range(4):
                                    nc.sync.dma_start(
                                        wout_sb[:, nwo, :], woutT.ap()[nwo])
                                    nwo += 1

                    while pend:
                        flush()
                    for t_ in norm_q:
                        t_()

                # ---------------- output projection (pure PE) ----------
                with tc.tile_pool(name=f"psC{b}", bufs=2, space="PSUM") as psc:
                    psc_pool[0] = psc
                    for hh in range(HPC):
                        for cpi in range(NCP):
                            outproj_chain(b, hh, cpi)

    nc.compile()
    return nc


_NC = None


def _get_nc():
    global _NC
    if _NC is None:
        _NC = _build()
    return _NC


def _host_tables():
    pos = np.arange(T, dtype=np.float32)[:, None]
    div = np.exp(np.arange(0, 2 * HALF, 2, dtype=np.float32)
                 * np.float32(-math.log(ROPE_BASE) / (2 * HALF)))
    ang = pos * div[None, :]
    cosv = np.cos(ang).astype(np.float32)   # [T, HALF]
    sinv = np.sin(ang).astype(np.float32)
    cosT = np.ascontiguousarray(cosv.T)     # [HALF, T]
    sinT = np.ascontiguousarray(sinv.T)
    cs2 = np.ascontiguousarray(
        np.concatenate([cosT, cosT], axis=0)).astype(ml_dtypes.bfloat16)
    sn1 = np.ascontiguousarray(sinT).astype(ml_dtypes.bfloat16)
    # triangle mask M[s, w] = 1 iff s <= w
    ww = np.arange(P)[None, :]
    ss = np.arange(P)[:, None]
    maskM = (ss <= ww).astype(ml_dtypes.bfloat16)
    return cs2, sn1, maskM


def _make_in_maps(x, Wqkv, Wout):
    x = np.asarray(x, dtype=np.float32)
    Wqkv = np.asarray(Wqkv, dtype=np.float32)
    Wout = np.asarray(Wout, dtype=np.float32)
    assert x.shape == (B, T, C) and Wqkv.shape == (C, 3 * C) \
        and Wout.shape == (C, C)

    cs2, sn1, maskM = _host_tables()
    # xTt[b, ti, p, ko, u] = x[b, ti*TQ+u, ko*128+p]
    xTt = np.ascontiguousarray(
        x.reshape(B, NT, TQ, KO, P).transpose(0, 1, 4, 3, 2)
    ).astype(ml_dtypes.bfloat16)
    woutT = np.ascontiguousarray(
        Wout.reshape(KO, P, C)).astype(ml_dtypes.bfloat16)

    in_maps = []
    for core in range(NCORES):
        h0 = core * HPC
        cols = slice(h0 * D, (h0 + HPC) * D)
        ws = []
        for part in range(3):
            w = Wqkv[:, part * C:(part + 1) * C][:, cols]  # [C, HPC*D]
            ws.append(np.ascontiguousarray(
                w.reshape(KO, P, HPC * D).transpose(1, 0, 2)
            ).astype(ml_dtypes.bfloat16))
        in_maps.append({
            "xTt": xTt,
            "wq": ws[0], "wk": ws[1], "wv": ws[2],
            "woutT": woutT,
            "cs2": cs2, "sn1": sn1, "maskM": maskM,
        })
    return in_maps


def _run(x, Wqkv, Wout, trace=False):
    nc = _get_nc()
    in_maps = _make_in_maps(x, Wqkv, Wout)
    res = run_bass_kernel_spmd(nc, in_maps, core_ids=list(range(NCORES)),
                               trace=trace)
    out = np.empty((B, T, C), dtype=np.float32)
    for core in range(NCORES):
        out[:, core * HPC * D:(core + 1) * HPC * D, :] = \
            res.results[core]["y"]
    return out, res


def kernel(x, Wqkv, Wout):
    out, _ = _run(x, Wqkv, Wout)
    return out


# revision 52
# speedup vs baseline: 1.0575x; 1.0008x over previous
"""Trainium2 Bass kernel for nn_MultiHeadAttention_63015760167496.

Computation (see reference): qkv = x @ Wqkv; RoPE on q,k; causal softmax
attention per head; out = einsum('bhts,bshd->bhtd', probs, v);
out.reshape(B,T,C) @ Wout  -- the reshape is a *head-major* flatten of
[B,H,T,D] into [B,T,C], so final-output row r = h*128 + t//16 depends only
on head h.  Sharding: head-parallel over 8 cores (2 heads/core); every core
computes its two heads end-to-end and produces final-output rows
[256*i, 256*i+256).  Host concatenates -- no collectives.

All on-device data is bf16 (PSUM accumulation f32), which halves DMA/SBUF
vs f32r at the same 1 cycle/row PE rate.  Attention runs in S^T layout
([s,t]): softmax denominator via a ones-column matmul (partition reduction
on the PE), normalization broadcast via gpsimd.partition_broadcast and a
flat VE multiply, both issued *deferred* (inside the next t-tile's block
loop) so they never gate the PE.  The attention inner loop is
software-pipelined: score blocks are processed in pairs sharing one
two-bank PSUM tile and a single exp instruction, and the PV/sum matmuls of
the previous two pairs are flushed in same-accumulation-group bursts (a
LDWEIGHTS after an accumulating matmul whose group is suspended stalls
~95ns on hw).  O^T is stored flat [d, t]; the out-projection reads it
through a strided LDWEIGHTS view.  Wout is prefetched into SBUF during
attention b=0 so the out-projection phases are pure PE.
"""

import math
import sys

for _p in ("/opt/trn_rl_repo", "/root/.axon_site/_ro/trn_rl_repo"):
    if _p not in sys.path:
        sys.path.insert(0, _p)

import numpy as np
import ml_dtypes

import concourse.bass as bass
import concourse.mybir as mybir
import concourse.tile as tile
from concourse import bacc
from concourse.bass_utils import run_bass_kernel_spmd

B, T, C = 2, 2048, 2048
H = 16            # heads total
D = C // H        # 128 head dim
HALF = D // 2     # 64
P = 128
KO = C // P       # 16 contraction chunks
NCORES = 8
HPC = H // NCORES  # 2 heads per core
TQ = 512          # t-tile for qkv projection
NT = T // TQ      # 4
TA = 512          # t-tile for attention
NTA = T // TA     # 4
NSC = T // P      # 16 s-chunks
ROPE_BASE = 10000.0
SCALE = 1.0 / math.sqrt(D)
TC_ = 512         # col-tile for out projection
NCP = C // TC_    # 4
LOOKAHEAD = 2     # attention software-pipeline depth

f32 = mybir.dt.float32
bf16 = mybir.dt.bfloat16
EXP = mybir.ActivationFunctionType.Exp


def _build():
    nc = bacc.Bacc("TRN2", target_bir_lowering=False, debug=False,
                   num_devices=NCORES)

    # host-pre-tiled x^T: xTt[b, ti, p, ko, u] = x[b, ti*TQ+u, ko*128+p]
    xTt = nc.dram_tensor("xTt", [B, NT, P, KO, TQ], bf16, kind="ExternalInput")
    # host-pre-chunked weights: w[p, ko, m] = W[ko*128+p, m]
    wq = nc.dram_tensor("wq", [P, KO, HPC * D], bf16, kind="ExternalInput")
    wk = nc.dram_tensor("wk", [P, KO, HPC * D], bf16, kind="ExternalInput")
    wv = nc.dram_tensor("wv", [P, KO, HPC * D], bf16, kind="ExternalInput")
    # woutT[j, p, c] = Wout[j*128+p, c]
    woutT = nc.dram_tensor("woutT", [KO, P, C], bf16, kind="ExternalInput")
    cs2 = nc.dram_tensor("cs2", [P, T], bf16, kind="ExternalInput")  # [cos;cos]
    sn1 = nc.dram_tensor("sn1", [HALF, T], bf16, kind="ExternalInput")  # sin
    maskM = nc.dram_tensor("maskM", [P, P], bf16, kind="ExternalInput")
    y = nc.dram_tensor("y", [B, HPC * D, C], f32, kind="ExternalOutput")

    with tile.TileContext(nc) as tc:
        with tc.tile_pool(name="const", bufs=1) as cp_, \
             tc.tile_pool(name="wo", bufs=1) as wop, \
             tc.tile_pool(name="qkv", bufs=1) as qp, \
             tc.tile_pool(name="ot", bufs=1) as op_, \
             tc.tile_pool(name="ys", bufs=4) as yp, \
             tc.tile_pool(name="small", bufs=2) as sp:

            wq_sb = cp_.tile([P, KO, HPC * D], bf16, tag="wq")
            wk_sb = cp_.tile([P, KO, HPC * D], bf16, tag="wk")
            wv_sb = cp_.tile([P, KO, HPC * D], bf16, tag="wv")
            cs_sb = cp_.tile([P, T], bf16, tag="cs")
            sn_sb = cp_.tile([HALF, T], bf16, tag="sn")
            mask_sb = cp_.tile([P, P], bf16, tag="mask")
            wout_sb = wop.tile([P, KO, C], bf16, tag="wout")

            # startup DMAs: wq first (chunked) so the first chain starts
            # ASAP; the first chunk goes through the gpsimd DGE (its
            # framework preamble ends ~2.5us before the sync engine's).
            nc.gpsimd.dma_start(wq_sb[:, 0:8, :], wq.ap()[:, 0:8, :])
            nc.sync.dma_start(wq_sb[:, 8:16, :], wq.ap()[:, 8:16, :])

            ones_f32 = cp_.tile([P, 1], f32, tag="ones_f32")
            nc.vector.memset(ones_f32[:], 1.0)
            ones_col = cp_.tile([P, 1], bf16, tag="ones_col")
            nc.vector.tensor_copy(ones_col[:], ones_f32[:])
            # act-table warmup: force the Exp table load at t=0 instead of
            # in the middle of the first attention block.
            warm_in = cp_.tile([1, 8], f32, tag="warm_in")
            nc.vector.memset(warm_in[:], 0.0)
            warm_out = cp_.tile([1, 8], f32, tag="warm_out")
            nc.scalar.activation(warm_out[:], warm_in[:], EXP, scale=1.0)

            # persistent attention outputs O^T per (b, local head): [d, t]
            oT = [[op_.tile([P, T], bf16, tag=f"oT{b}{hh}", name=f"oT{b}{hh}")
                   for hh in range(HPC)] for b in range(B)]

            def outproj_chain(b, hh, cpi):
                csl = slice(cpi * TC_, (cpi + 1) * TC_)
                psy = psc_pool[0].tile([P, TC_], f32, tag="y")
                # stationary: oT columns {t : t%16 == j}, strided view
                ovw = oT[b][hh].rearrange("p (u j) -> p j u", j=KO)
                for j in range(KO):
                    nc.tensor.matmul(psy[:], ovw[:, j, :],
                                     wout_sb[:, j, csl],
                                     start=(j == 0), stop=(j == KO - 1))
                ysb = yp.tile([P, TC_], f32, tag="ysb")
                nc.scalar.copy(ysb[:], psy[:])
                nc.sync.dma_start(
                    y.ap()[b, hh * D:(hh + 1) * D, csl], ysb[:])

            psc_pool = [None]

            for b in range(B):
                qT = [qp.tile([P, T], bf16, tag=f"qT{hh}", name=f"qT{b}{hh}")
                      for hh in range(HPC)]
                kT = [qp.tile([P, T], bf16, tag=f"kT{hh}", name=f"kT{b}{hh}")
                      for hh in range(HPC)]
                vt = [qp.tile([P, NSC, D], bf16, tag=f"v{hh}", name=f"v{b}{hh}")
                      for hh in range(HPC)]

                # ---------------- QKV projection + RoPE ----------------
                with tc.tile_pool(name=f"xt{b}", bufs=3) as xp, \
                     tc.tile_pool(name=f"psA{b}", bufs=3, space="PSUM") as psa, \
                     tc.tile_pool(name=f"psV{b}", bufs=2, space="PSUM") as psv_p, \
                     tc.tile_pool(name=f"rope{b}", bufs=3) as rp:

                    def qkmm(xt, w_sb, hh, nm):
                        hsl = slice(hh * D, (hh + 1) * D)
                        ps = psa.tile([P, TQ], f32, tag="acc", name=nm)
                        for ko in range(KO):
                            nc.tensor.matmul(ps[:], w_sb[:, ko, hsl],
                                             xt[:, ko, :],
                                             start=(ko == 0),
                                             stop=(ko == KO - 1))
                        return ps

                    def rope(ps, dst, sl):
                        # tcos = ps * [cos;cos]; tsw pre-swaps halves:
                        # tsw[0:64]=q2*sin, tsw[64:128]=q1*sin so the add/sub
                        # reads align on base partitions.  All elementwise
                        # work on the VE (bf16 operands get 2x mode).
                        cs = cs_sb[:, sl]
                        sn = sn_sb[:, sl]
                        tcos = rp.tile([P, TQ], bf16, tag="tcos")
                        tsw = rp.tile([P, TQ], bf16, tag="tsw")
                        nc.vector.tensor_mul(tcos[:], ps[:], cs)
                        nc.vector.tensor_mul(tsw[0:HALF, :], ps[HALF:P, :], sn)
                        nc.vector.tensor_mul(tsw[HALF:P, :], ps[0:HALF, :], sn)
                        nc.vector.tensor_sub(dst[0:HALF, sl],
                                             tcos[0:HALF, :], tsw[0:HALF, :])
                        nc.vector.tensor_add(dst[HALF:P, sl],
                                             tcos[HALF:P, :], tsw[HALF:P, :])

                    def vchain(xt, ti):
                        for sub in range(TQ // P):
                            psv = psv_p.tile([P, HPC * D], f32, tag="acc")
                            for ko in range(KO):
                                nc.tensor.matmul(
                                    psv[:], xt[:, ko, sub * P:(sub + 1) * P],
                                    wv_sb[:, ko, :],
                                    start=(ko == 0), stop=(ko == KO - 1))
                            tci = ti * (TQ // P) + sub
                            for hh in range(HPC):
                                # Act engine is idle during QKV; it does the
                                # psum->sbuf v copies.
                                nc.scalar.copy(
                                    vt[hh][:, tci, :],
                                    psv[:, hh * D:(hh + 1) * D])

                    xts = {}
                    for ti in range(NT):
                        xts[ti] = xp.tile([P, KO, TQ], bf16, tag="xt",
                                          name=f"xt{b}_{ti}")

                    if b == 0:
                        # Startup is a DMA-bandwidth wall: ~7MB must land in
                        # the first ~30us.  Chunk the first two x tiles so
                        # chains pace behind arriving data, interleave wq/x
                        # chunks in ko-consumption order, and defer ti0's
                        # v-chains until after ti1's q/k so wv is needed
                        # later.
                        for g in range(4):
                            nc.sync.dma_start(
                                xts[0][:, 4 * g:4 * g + 4, :],
                                xTt.ap()[b, 0, :, 4 * g:4 * g + 4, :])
                        ps = qkmm(xts[0], wq_sb, 0, "acc0_q0")
                        nc.sync.dma_start(wk_sb[:], wk.ap())
                        nc.sync.dma_start(cs_sb[:], cs2.ap())
                        nc.sync.dma_start(sn_sb[:], sn1.ap())
                        rope(ps, qT[0], slice(0, TQ))
                        rope(qkmm(xts[0], wq_sb, 1, "acc0_q1"), qT[1],
                             slice(0, TQ))
                        for g in range(4):
                            nc.sync.dma_start(
                                xts[1][:, 4 * g:4 * g + 4, :],
                                xTt.ap()[b, 1, :, 4 * g:4 * g + 4, :])
                        rope(qkmm(xts[0], wk_sb, 0, "acc0_k0"), kT[0],
                             slice(0, TQ))
                        nc.sync.dma_start(wv_sb[:], wv.ap())
                        nc.sync.dma_start(mask_sb[:], maskM.ap())
                        rope(qkmm(xts[0], wk_sb, 1, "acc0_k1"), kT[1],
                             slice(0, TQ))
                        sl1 = slice(TQ, 2 * TQ)
                        rope(qkmm(xts[1], wq_sb, 0, "acc1_q0"), qT[0], sl1)
                        rope(qkmm(xts[1], wq_sb, 1, "acc1_q1"), qT[1], sl1)
                        rope(qkmm(xts[1], wk_sb, 0, "acc1_k0"), kT[0], sl1)
                        rope(qkmm(xts[1], wk_sb, 1, "acc1_k1"), kT[1], sl1)
                        vchain(xts[1], 1)
                        vchain(xts[0], 0)
                        rest = range(2, NT)
                    else:
                        rest = range(NT)

                    for ti in rest:
                        sl = slice(ti * TQ, (ti + 1) * TQ)
                        xt = xts[ti]
                        nc.sync.dma_start(xt[:], xTt.ap()[b, ti])
                        for hh in range(HPC):
                            rope(qkmm(xt, wq_sb, hh, f"a{ti}q{hh}"),
                                 qT[hh], sl)
                            rope(qkmm(xt, wk_sb, hh, f"a{ti}k{hh}"),
                                 kT[hh], sl)
                        vchain(xt, ti)

                # ------------- attention (S^T layout) + interleaved -----
                # ------------- out-projection of the previous head ------
                # s-chunks are processed in PAIRS sharing one 2-bank PSUM
                # tile and a single exp instruction, so the Act engine
                # (1024 cols + one fixed overhead) runs faster than the
                # PE's 6 matmuls per pair and never paces the pipeline.
                with tc.tile_pool(name=f"psBsc{b}", bufs=2, space="PSUM") as pssc, \
                     tc.tile_pool(name=f"psBo{b}", bufs=2, space="PSUM") as pso, \
                     tc.tile_pool(name=f"psBsum{b}", bufs=2, space="PSUM") as pssum, \
                     tc.tile_pool(name=f"pt{b}", bufs=5) as ptp, \
                     tc.tile_pool(name=f"pts{b}", bufs=6) as ptsp:
                    nwo = 0   # wout prefetch cursor (b == 0 only)
                    # Deferred-issue queue: each t-tile's normalize is
                    # issued inside the NEXT tile's block loop (ps_o and
                    # ps_sum have bufs=2, so it must be issued before the
                    # slot cycles) -- the PE/Act pipeline never waits on it.
                    norm_q = []
                    # the score->PV pipeline is carried ACROSS (head,
                    # t-tile) boundaries: the next tile's score matmuls
                    # cover the previous tile's final flush, so no tile
                    # drains with an exp-wait bubble.  pend entries carry
                    # their own (ps_o, ps_sum, head, is-last-pair) context.
                    pend = []

                    def flush():
                        # same-accumulation-group matmuls must be adjacent:
                        # a LDWEIGHTS that follows an accumulating matmul
                        # whose group is being suspended stalls ~95ns on hw
                        # (after a STOPPED group it is free).  Flush up to
                        # TWO pairs at once, all o-matmuls in one burst
                        # then all sum-matmuls.  Non-diag pairs carry a
                        # VE-pre-added pair-sum tile, so their softmax
                        # denominator needs ONE ones-matmul, not two.
                        take, pend[:] = pend[:2], pend[2:]
                        for (pt_, sws, ps_o_, ps_sum_, hh_, lastp, pts,
                             ptq, dg) in take:
                            for k, (s_, w_) in enumerate(sws):
                                nc.tensor.matmul(
                                    ps_o_[:, w_], vt[hh_][:, s_, :],
                                    pt_[:, k, w_], start=(s_ == 0),
                                    stop=(lastp and k == len(sws) - 1))
                        # flush pops are (even, odd)-pair aligned within a
                        # tile (every tile has an even pair count), so a
                        # quad-sum precomputed on the odd entry (ready two
                        # pairs early, no VE wait) covers the whole take
                        # with ONE ones-matmul.
                        if (len(take) == 2 and take[1][7] is not None
                                and take[1][3] is take[0][3]):
                            nc.tensor.matmul(
                                take[0][3][:, :], ones_col[:],
                                take[1][7][:],
                                start=(take[0][1][0][0] == 0), stop=False)
                            return
                        for (pt_, sws, ps_o_, ps_sum_, hh_, lastp, pts,
                             ptq, dg) in take:
                            first = sws[0][0] == 0
                            if pts is not None:
                                nc.tensor.matmul(
                                    ps_sum_[:, :], ones_col[:], pts[:],
                                    start=first, stop=False)
                                continue
                            if dg is not None and not first:
                                # both accumulate (start would zero the
                                # whole 2KB bank region, clobbering the
                                # other sub-range's partial sums)
                                ptd, lo, ov = dg
                                nc.tensor.matmul(
                                    ps_sum_[:, ov:TA], ones_col[:],
                                    ptd[:, ov:TA],
                                    start=False, stop=False)
                                nc.tensor.matmul(
                                    ps_sum_[:, lo:ov], ones_col[:],
                                    pt_[:, 0, lo:ov],
                                    start=False, stop=lastp)
                                continue
                            for k, (s_, w_) in enumerate(sws):
                                nc.tensor.matmul(
                                    ps_sum_[:, w_], ones_col[:],
                                    pt_[:, k, w_], start=(s_ == 0),
                                    stop=(lastp and k == len(sws) - 1))

                    for hh in range(HPC):
                        for ta in range(NTA):
                            spt = TA // P
                            tsl = slice(ta * TA, (ta + 1) * TA)
                            ps_o = pso.tile([P, TA], f32, tag="o")
                            ps_sum = pssum.tile([1, TA], f32, tag="sum")
                            nblk = (ta + 1) * spt
                            prev_pts = None

                            for pi in range(nblk // 2):
                                ps_sc = pssc.tile([P, 2, TA], f32, tag="sc")
                                pt = ptp.tile([P, 2, TA], bf16, tag="pt")
                                sws = []
                                for k in range(2):
                                    s = 2 * pi + k
                                    diag = s >= ta * spt
                                    t_lo = (s - ta * spt) * P if diag else 0
                                    w = slice(t_lo, TA)
                                    qsl = slice(ta * TA + t_lo,
                                                (ta + 1) * TA)
                                    nc.tensor.matmul(
                                        ps_sc[:, k, w],
                                        kT[hh][:, s * P:(s + 1) * P],
                                        qT[hh][:, qsl],
                                        start=True, stop=True)
                                    sws.append((s, w))
                                # one exp for both chunks; cols outside a
                                # diag chunk's window hold stale psum ->
                                # garbage pt that no matmul reads
                                nc.scalar.activation(pt[:, :, :],
                                                     ps_sc[:, :, :],
                                                     EXP, scale=SCALE)
                                for k, (s, w) in enumerate(sws):
                                    if s >= ta * spt:  # mask the triangle
                                        t_lo = (s - ta * spt) * P
                                        nc.vector.tensor_mul(
                                            pt[:, k, t_lo:t_lo + P],
                                            pt[:, k, t_lo:t_lo + P],
                                            mask_sb[:])
                                pts = None
                                ptq = None
                                dg = None
                                if pi < 2 * ta:  # non-diag pair
                                    pts = ptsp.tile([P, TA], bf16,
                                                    tag="ptsum")
                                    nc.vector.tensor_add(pts[:],
                                                         pt[:, 0, :],
                                                         pt[:, 1, :])
                                    if pi % 2 == 1:
                                        # quad-sum of pairs (pi-1, pi),
                                        # ready well before its flush
                                        ptq = ptsp.tile([P, TA], bf16,
                                                        tag="ptsum")
                                        nc.vector.tensor_add(
                                            ptq[:], prev_pts[:], pts[:])
                                else:
                                    # diag pair: chunks are valid on
                                    # [lo, TA) and [lo+128, TA); VE-add
                                    # the (post-mask) overlap so the sum
                                    # needs one merged ones-matmul plus a
                                    # 128-wide strip from chunk A.
                                    lo = sws[0][1].start
                                    ov = lo + P
                                    ptd = ptsp.tile([P, TA], bf16,
                                                    tag="ptsum")
                                    nc.vector.tensor_add(
                                        ptd[:, ov:TA], pt[:, 0, ov:TA],
                                        pt[:, 1, ov:TA])
                                    dg = (ptd, lo, ov)
                                prev_pts = pts
                                pend.append((pt, sws, ps_o, ps_sum, hh,
                                             pi == nblk // 2 - 1, pts,
                                             ptq, dg))
                                if len(pend) > 3:
                                    flush()
                                # drain at pi==1: with lag 3 the previous
                                # tile's stop-matmuls only issue in the
                                # pi1 flush above; the normalize must
                                # trace after them.
                                if pi == 1 and norm_q:
                                    norm_q.pop(0)()

                            # normalization, deferred: recip on VE,
                            # partition-broadcast on gpsimd, flat multiply
                            # on VE -- issued inside the next tile's block
                            # loop so the PE/Act pipeline never waits.
                            def normalize(ps_o=ps_o, ps_sum=ps_sum,
                                          hh=hh, tsl=tsl):
                                recf = sp.tile([1, TA], f32, tag="recf")
                                nc.vector.reciprocal_approx_fast(
                                    recf[:], ps_sum[:])
                                recb = sp.tile([1, TA], bf16, tag="recb")
                                nc.vector.tensor_copy(recb[:], recf[:])
                                bcb = sp.tile([P, TA], bf16, tag="bcb")
                                nc.gpsimd.partition_broadcast(bcb[:],
                                                              recb[:],
                                                              channels=P)
                                nc.vector.tensor_mul(oT[b][hh][:, tsl],
                                                     ps_o[:], bcb[:])
                            norm_q.append(normalize)

                            if b == 0 and hh == 0:
                                # prefetch all of wout during head 0's
                                # attention (the first out-proj chain needs
                                # every j block)
                                for _ in range(4):
                                    nc.sync.dma_start(
                                        wout_sb[:, nwo, :], woutT.ap()[nwo])
                                    nwo += 1

                    while pend:
                        flush()
                    for t_ in norm_q:
                        t_()

                # ---------------- output projection (pure PE) ----------
                with tc.tile_pool(name=f"psC{b}", bufs=2, space="PSUM") as psc:
                    psc_pool[0] = psc
                    for hh in range(HPC):
                        for cpi in range(NCP):
                            outproj_chain(b, hh, cpi)

    nc.compile()
    return nc


_NC = None


def _get_nc():
    global _NC
    if _NC is None:
        _NC = _build()
    return _NC


def _host_tables():
    pos = np.arange(T, dtype=np.float32)[:, None]
    div = np.exp(np.arange(0, 2 * HALF, 2, dtype=np.float32)
                 * np.float32(-math.log(ROPE_BASE) / (2 * HALF)))
    ang = pos * div[None, :]
    cosv = np.cos(ang).astype(np.float32)   # [T, HALF]
    sinv = np.sin(ang).astype(np.float32)
    cosT = np.ascontiguousarray(cosv.T)     # [HALF, T]
    sinT = np.ascontiguousarray(sinv.T)
    cs2 = np.ascontiguousarray(
        np.concatenate([cosT, cosT], axis=0)).astype(ml_dtypes.bfloat16)
    sn1 = np.ascontiguousarray(sinT).astype(ml_dtypes.bfloat16)
    # triangle mask M[s, w] = 1 iff s <= w
    ww = np.arange(P)[None, :]
    ss = np.arange(P)[:, None]
    maskM = (ss <= ww).astype(ml_dtypes.bfloat16)
    return cs2, sn1, maskM


def _make_in_maps(x, Wqkv, Wout):
    x = np.asarray(x, dtype=np.float32)
    Wqkv = np.asarray(Wqkv, dtype=np.float32)
    Wout = np.asarray(Wout, dtype=np.float32)
    assert x.shape == (B, T, C) and Wqkv.shape == (C, 3 * C) \
        and Wout.shape == (C, C)

    cs2, sn1, maskM = _host_tables()
    # xTt[b, ti, p, ko, u] = x[b, ti*TQ+u, ko*128+p]
    xTt = np.ascontiguousarray(
        x.reshape(B, NT, TQ, KO, P).transpose(0, 1, 4, 3, 2)
    ).astype(ml_dtypes.bfloat16)
    woutT = np.ascontiguousarray(
        Wout.reshape(KO, P, C)).astype(ml_dtypes.bfloat16)

    in_maps = []
    for core in range(NCORES):
        h0 = core * HPC
        cols = slice(h0 * D, (h0 + HPC) * D)
        ws = []
        for part in range(3):
            w = Wqkv[:, part * C:(part + 1) * C][:, cols]  # [C, HPC*D]
            ws.append(np.ascontiguousarray(
                w.reshape(KO, P, HPC * D).transpose(1, 0, 2)
            ).astype(ml_dtypes.bfloat16))
        in_maps.append({
            "xTt": xTt,
            "wq": ws[0], "wk": ws[1], "wv": ws[2],
            "woutT": woutT,
            "cs2": cs2, "sn1": sn1, "maskM": maskM,
        })
    return in_maps


def _run(x, Wqkv, Wout, trace=False):
    nc = _get_nc()
    in_maps = _make_in_maps(x, Wqkv, Wout)
    res = run_bass_kernel_spmd(nc, in_maps, core_ids=list(range(NCORES)),
                               trace=trace)
    out = np.empty((B, T, C), dtype=np.float32)
    for core in range(NCORES):
        out[:, core * HPC * D:(core + 1) * HPC * D, :] = \
            res.results[core]["y"]
    return out, res


def kernel(x, Wqkv, Wout):
    out, _ = _run(x, Wqkv, Wout)
    return out
